# revision 7
# baseline (speedup 1.0000x reference)
"""Trainium2 Bass kernel for nn_BiInteraction (segment softmax bi-interaction).

Strategy (data-parallel over molecules, 8 NeuronCores):
  - Each core owns 8 molecules (its contiguous slice of the batch) and gets:
    its slice of protSeq_embed in two layouts (transposed on host for the
    score matmul; natural layout is rebuilt on-chip via PE transposes for the
    attention pool), its atoms padded to 64 slots per molecule (pads are
    REPLICAS of a real atom, which makes every max reduction exact without
    masks), an indicator matrix for segment sums, and the replicated MLP
    weights.
  - Scores S[a, l] = (atom @ W_att) . prot[seg(a), l] are computed
    block-diagonally: one matmul per molecule (k = d = 128 contraction).
  - Segment softmax over atoms:   Wc = exp(max_l S); Sc = 1^T (ind * Wc).
  - Residue softmax over protein: Wp = max_a S via PE transpose + grouped
    reduce; ew = exp(Wp); t = sum_l ew via ones-matmul.
  - Pools via matmuls (atoms / residues on the contraction partitions),
    normalization via a k=1 broadcast matmul + reciprocal, then the 3-layer
    MLP entirely on-chip per core ([256]->512->256->1 for its 8 molecules).

All shapes are static and identical across cores (single SPMD program);
per-core differences (counts, indicators, padding) live in the DMA'd data.
"""

import numpy as np

import concourse.bacc as bacc
import concourse.bass as bass
import concourse.tile as tile
from concourse import mybir
from concourse.bass_utils import run_bass_kernel_spmd

F32 = mybir.dt.float32
AxX = mybir.AxisListType.X
AF = mybir.ActivationFunctionType

A, L, D, B = 2048, 512, 128, 64
H1, H2 = 512, 256
NCORES = 8
MPC = B // NCORES            # molecules per core = 8
NPAD = 64                    # padded atom slots per molecule
NSTACK = MPC * NPAD // 128   # stacks of 128 padded atoms per core = 4

# consts tensor column layout
C_IDENT = 0        # [0, 128)   identity
C_IND = 128        # [128, 136) indicator, col = molecule
C_ONES = 136       # [136, 137) ones column
C_B1 = 137         # [137, 141) b1 chunks
C_B2 = 141         # [141, 143) b2 chunks
C_WO = 143         # [143, 145) Wo chunks
C_W = 145

_PROGRAM_CACHE = {}


def _build_program():
    nc = bacc.Bacc("TRN2", target_bir_lowering=False, debug=False)

    d_protT = nc.dram_tensor("protT", [128, MPC * L], F32, kind="ExternalInput")
    d_atomT = nc.dram_tensor("atomT", [128, MPC * NPAD], F32, kind="ExternalInput")
    d_atomN = nc.dram_tensor("atomN", [128, NSTACK * D], F32, kind="ExternalInput")
    d_watt = nc.dram_tensor("watt", [128, D], F32, kind="ExternalInput")
    d_w1 = nc.dram_tensor("w1", [128, 2 * H1], F32, kind="ExternalInput")
    d_w2 = nc.dram_tensor("w2", [128, 4 * H2], F32, kind="ExternalInput")
    d_consts = nc.dram_tensor("consts", [128, C_W], F32, kind="ExternalInput")
    d_row = nc.dram_tensor("row", [1, 129], F32, kind="ExternalInput")
    d_y = nc.dram_tensor("y", [MPC, 1], F32, kind="ExternalOutput")

    with tile.TileContext(nc) as tc:
        with (
            tc.tile_pool(name="weights", bufs=1) as wpool,
            tc.tile_pool(name="work", bufs=1) as work,
            tc.tile_pool(name="spool", bufs=2) as spool,
            tc.tile_pool(name="psum_big", bufs=3, space=bass.MemorySpace.PSUM) as pbig,
            tc.tile_pool(name="psum_q", bufs=2, space=bass.MemorySpace.PSUM) as pq,
            tc.tile_pool(name="psum_s", bufs=3, space=bass.MemorySpace.PSUM) as ps,
        ):
            # ---- loads -------------------------------------------------
            protT = wpool.tile([128, MPC * L], F32)
            nc.sync.dma_start(protT[:], d_protT[:])
            atomT = wpool.tile([128, MPC * NPAD], F32)
            nc.sync.dma_start(atomT[:], d_atomT[:])
            atomN = wpool.tile([128, NSTACK, D], F32)
            nc.sync.dma_start(atomN[:], d_atomN[:])
            watt = wpool.tile([128, D], F32)
            nc.sync.dma_start(watt[:], d_watt[:])
            w1 = wpool.tile([128, 2 * H1], F32)
            nc.sync.dma_start(w1[:], d_w1[:])
            w2 = wpool.tile([128, 4 * H2], F32)
            nc.sync.dma_start(w2[:], d_w2[:])
            consts = wpool.tile([128, C_W], F32)
            nc.sync.dma_start(consts[:], d_consts[:])
            row = wpool.tile([1, 129], F32)
            nc.sync.dma_start(row[:], d_row[:])

            ident = consts[:, C_IDENT : C_IDENT + 128]
            ones_col = consts[:, C_ONES : C_ONES + 1]

            # ---- XT = W_att.T-applied atoms: XT[d', a] -----------------
            ps_xt = pbig.tile([128, MPC * NPAD], F32, tag="big")
            nc.tensor.matmul(ps_xt[:], watt[:], atomT[:], start=True, stop=True)
            xt = work.tile([128, MPC * NPAD], F32)
            nc.vector.tensor_copy(xt[:], ps_xt[:])

            # ---- P_nat via PE transposes of protT chunks ---------------
            pnat = work.tile([128, MPC * 4 * 128], F32)  # col block (m, j)
            for t in range(MPC * 4):
                m, j = divmod(t, 4)
                ps_tr = pq.tile([128, 128], F32, tag="q")
                nc.tensor.transpose(
                    ps_tr[:], protT[:, m * L + j * 128 : m * L + (j + 1) * 128], ident
                )
                if t % 2 == 0:
                    nc.scalar.copy(pnat[:, t * 128 : (t + 1) * 128], ps_tr[:])
                else:
                    nc.vector.tensor_copy(pnat[:, t * 128 : (t + 1) * 128], ps_tr[:])

            # ---- scores: S[a, l] per molecule, stacked 2/psum bank -----
            s_psums = []
            for s in range(NSTACK):
                ps_S = pbig.tile([128, L], F32, tag="big")
                s_psums.append(ps_S)
                for slot in range(2):
                    i = 2 * s + slot
                    nc.tensor.matmul(
                        ps_S[slot * NPAD : (slot + 1) * NPAD, :],
                        xt[:, i * NPAD : (i + 1) * NPAD],
                        protT[:, i * L : (i + 1) * L],
                        start=True,
                        stop=True,
                    )

            # ---- Wc = exp(max_l S) per atom ----------------------------
            wc = work.tile([128, NSTACK], F32)
            for s in range(NSTACK):
                nc.vector.reduce_max(wc[:, s : s + 1], s_psums[s][:], axis=AxX)
            wce = work.tile([128, NSTACK], F32)
            nc.scalar.activation(wce[:], wc[:], AF.Exp)

            # ---- S to SBUF; transposes; Wp = max_a S -------------------
            wp = work.tile([128, 4 * MPC], F32)  # col = 8*j + m
            for s in range(NSTACK):
                s_sb = spool.tile([128, L], F32, tag="s_sb")
                if s % 2 == 0:
                    nc.scalar.copy(s_sb[:], s_psums[s][:])
                else:
                    nc.vector.tensor_copy(s_sb[:], s_psums[s][:])
                for j in range(4):
                    ps_st = pq.tile([128, 128], F32, tag="q")
                    nc.tensor.transpose(
                        ps_st[:], s_sb[:, j * 128 : (j + 1) * 128], ident
                    )
                    nc.vector.reduce_max(
                        wp[:, 8 * j + 2 * s : 8 * j + 2 * s + 2],
                        ps_st[:].rearrange("p (g k) -> p g k", k=NPAD),
                        axis=AxX,
                    )
            ew = work.tile([128, 4 * MPC], F32)
            nc.scalar.activation(ew[:], wp[:], AF.Exp)

            # ---- denominators: Sc and t --------------------------------
            wcseg = work.tile([128, MPC], F32)
            for s in range(NSTACK):
                nc.vector.tensor_scalar_mul(
                    wcseg[:, 2 * s : 2 * s + 2],
                    in0=consts[:, C_IND + 2 * s : C_IND + 2 * s + 2],
                    scalar1=wce[:, s : s + 1],
                )
            ps_sc = ps.tile([1, MPC], F32, tag="sp")
            nc.tensor.matmul(ps_sc[:], ones_col, wcseg[:], start=True, stop=True)

            tpart = work.tile([128, MPC], F32)
            nc.vector.reduce_sum(
                tpart[:], ew[:].rearrange("p (j m) -> p m j", m=MPC), axis=AxX
            )
            ps_t = ps.tile([1, MPC], F32, tag="sp")
            nc.tensor.matmul(ps_t[:], ones_col, tpart[:], start=True, stop=True)

            sct = work.tile([1, 2 * MPC], F32)
            nc.vector.tensor_copy(sct[:, :MPC], ps_sc[:])
            nc.vector.tensor_copy(sct[:, MPC:], ps_t[:])
            ps_bc = ps.tile([128, 2 * MPC], F32, tag="sp")
            nc.tensor.matmul(ps_bc[:], row[:, :128], sct[:], start=True, stop=True)
            inv = work.tile([128, 2 * MPC], F32)
            nc.vector.reciprocal(inv[:], ps_bc[:])

            # ---- pools -------------------------------------------------
            ps_ap = ps.tile([128, MPC], F32, tag="sp")
            for s in range(NSTACK):
                nc.tensor.matmul(
                    ps_ap[:, 2 * s : 2 * s + 2],
                    atomN[:, s, :],
                    wcseg[:, 2 * s : 2 * s + 2],
                    start=True,
                    stop=True,
                )
            ps_pp = ps.tile([128, MPC], F32, tag="sp")
            for m in range(MPC):
                for j in range(4):
                    nc.tensor.matmul(
                        ps_pp[:, m : m + 1],
                        pnat[:, (4 * m + j) * 128 : (4 * m + j + 1) * 128],
                        ew[:, 8 * j + m : 8 * j + m + 1],
                        start=(j == 0),
                        stop=(j == 3),
                    )

            htop = work.tile([128, MPC], F32)
            nc.vector.tensor_mul(htop[:], ps_ap[:], inv[:, :MPC])
            hbot = work.tile([128, MPC], F32)
            nc.vector.tensor_mul(hbot[:], ps_pp[:], inv[:, MPC:])

            # ---- MLP ---------------------------------------------------
            h1 = work.tile([128, 4 * MPC], F32)
            for mc in range(4):
                ps_h1 = ps.tile([128, MPC], F32, tag="sp")
                nc.tensor.matmul(
                    ps_h1[:],
                    w1[:, mc * 128 : (mc + 1) * 128],
                    htop[:],
                    start=True,
                    stop=False,
                )
                nc.tensor.matmul(
                    ps_h1[:],
                    w1[:, H1 + mc * 128 : H1 + (mc + 1) * 128],
                    hbot[:],
                    start=False,
                    stop=True,
                )
                nc.scalar.activation(
                    h1[:, mc * MPC : (mc + 1) * MPC],
                    ps_h1[:],
                    AF.Relu,
                    bias=consts[:, C_B1 + mc : C_B1 + mc + 1],
                )
            h2 = work.tile([128, 2 * MPC], F32)
            for mc2 in range(2):
                ps_h2 = ps.tile([128, MPC], F32, tag="sp")
                for kc in range(4):
                    nc.tensor.matmul(
                        ps_h2[:],
                        w2[:, kc * H2 + mc2 * 128 : kc * H2 + (mc2 + 1) * 128],
                        h1[:, kc * MPC : (kc + 1) * MPC],
                        start=(kc == 0),
                        stop=(kc == 3),
                    )
                nc.scalar.activation(
                    h2[:, mc2 * MPC : (mc2 + 1) * MPC],
                    ps_h2[:],
                    AF.Relu,
                    bias=consts[:, C_B2 + mc2 : C_B2 + mc2 + 1],
                )
            ps_o = ps.tile([MPC, 1], F32, tag="sp")
            nc.tensor.matmul(
                ps_o[:], h2[:, :MPC], consts[:, C_WO : C_WO + 1], start=True, stop=False
            )
            nc.tensor.matmul(
                ps_o[:],
                h2[:, MPC : 2 * MPC],
                consts[:, C_WO + 1 : C_WO + 2],
                start=False,
                stop=False,
            )
            nc.tensor.matmul(
                ps_o[:], row[:, :MPC], row[:, 128:129], start=False, stop=True
            )
            y_sb = work.tile([MPC, 1], F32)
            nc.vector.tensor_copy(y_sb[:], ps_o[:])
            nc.sync.dma_start(d_y[:], y_sb[:])

    nc.compile()
    return nc


def _prep_inputs(atom_embed, protSeq_embed, atom_splits, W_att, W1, b1, W2, b2, Wo, bo):
    atom = np.ascontiguousarray(np.asarray(atom_embed, dtype=np.float32))
    prot = np.ascontiguousarray(np.asarray(protSeq_embed, dtype=np.float32))
    splits = np.asarray(atom_splits).astype(np.int64).ravel()
    order = np.argsort(splits, kind="stable")
    counts = np.bincount(splits, minlength=B)
    assert counts.max() <= NPAD, f"molecule with {counts.max()} atoms > NPAD={NPAD}"
    assert counts.min() >= 1, "empty molecule (reference produces NaN there)"
    offs = np.concatenate([[0], np.cumsum(counts)])

    atomP = np.empty((B, NPAD, D), np.float32)
    ind = np.zeros((B, NPAD), np.float32)
    for b in range(B):
        idx = order[offs[b] : offs[b + 1]]
        n = len(idx)
        atomP[b, :n] = atom[idx]
        atomP[b, n:] = atom[idx[0]]  # replicate a real atom: maxes stay exact
        ind[b, :n] = 1.0

    w_att = np.ascontiguousarray(np.asarray(W_att, np.float32))
    w1h = np.ascontiguousarray(
        np.asarray(W1, np.float32).reshape(2, 128, H1).transpose(1, 0, 2).reshape(128, 2 * H1)
    )
    w2h = np.ascontiguousarray(
        np.asarray(W2, np.float32).reshape(4, 128, H2).transpose(1, 0, 2).reshape(128, 4 * H2)
    )
    b1c = np.asarray(b1, np.float32).reshape(4, 128).T
    b2c = np.asarray(b2, np.float32).reshape(2, 128).T
    woc = np.asarray(Wo, np.float32).reshape(2, 128).T
    row = np.zeros((1, 129), np.float32)
    row[0, :128] = 1.0
    row[0, 128] = np.asarray(bo, np.float32).ravel()[0]

    in_maps = []
    for c in range(NCORES):
        sl = slice(c * MPC, (c + 1) * MPC)
        protT_c = np.ascontiguousarray(
            prot[sl].transpose(2, 0, 1).reshape(128, MPC * L)
        )
        atomT_c = np.ascontiguousarray(atomP[sl].reshape(MPC * NPAD, D).T)
        atomN_c = np.ascontiguousarray(
            atomP[sl].reshape(NSTACK, 128, D).transpose(1, 0, 2).reshape(128, NSTACK * D)
        )
        ind_c = np.zeros((128, MPC), np.float32)
        for m in range(MPC):
            s, slot = divmod(m, 2)
            ind_c[slot * NPAD : (slot + 1) * NPAD, m] = ind[c * MPC + m]
        consts = np.zeros((128, C_W), np.float32)
        consts[:, C_IDENT : C_IDENT + 128] = np.eye(128, dtype=np.float32)
        consts[:, C_IND : C_IND + MPC] = ind_c
        consts[:, C_ONES] = 1.0
        consts[:, C_B1 : C_B1 + 4] = b1c
        consts[:, C_B2 : C_B2 + 2] = b2c
        consts[:, C_WO : C_WO + 2] = woc
        in_maps.append(
            {
                "protT": protT_c,
                "atomT": atomT_c,
                "atomN": atomN_c,
                "watt": w_att,
                "w1": w1h,
                "w2": w2h,
                "consts": consts,
                "row": row,
            }
        )
    return in_maps


def kernel(atom_embed, protSeq_embed, atom_splits, W_att, W1, b1, W2, b2, Wo, bo,
           _trace=False):
    if "nc" not in _PROGRAM_CACHE:
        _PROGRAM_CACHE["nc"] = _build_program()
    nc = _PROGRAM_CACHE["nc"]
    in_maps = _prep_inputs(
        atom_embed, protSeq_embed, atom_splits, W_att, W1, b1, W2, b2, Wo, bo
    )
    res = run_bass_kernel_spmd(
        nc, in_maps, core_ids=list(range(NCORES)), trace=_trace
    )
    _PROGRAM_CACHE["last_result"] = res
    out = np.concatenate([res.results[c]["y"] for c in range(NCORES)], axis=0)
    return out.astype(np.float32)


# revision 11
# speedup vs baseline: 2.1366x; 2.1366x over previous
"""Trainium2 Bass kernel for nn_BiInteraction (segment softmax bi-interaction).

Strategy (data-parallel over molecules, 8 NeuronCores):
  - Each core owns 8 molecules (its contiguous slice of the batch) and gets
    its slice of protSeq_embed in two layouts (host-transposed protT for the
    score matmuls; natural-layout chunks for the attention pool), its atoms
    padded to 64 slots per molecule (pads are REPLICAS of a real atom, which
    keeps every max reduction exact without masks), an indicator matrix for
    segment sums, and the replicated MLP weights.
  - All matmul operands are fp16 (PSUM accumulation stays fp32): single-pass
    PE matmuls (fp32 runs LOW_HIGH double-pass at 4x the cost) and half the
    HBM traffic. End-to-end error vs the fp32 reference is ~1e-3.
  - Scores S[a, l] = (atom @ W_att) . prot[seg(a), l] are computed
    block-diagonally: one matmul per molecule (k = d = 128 contraction),
    two molecules stacked per PSUM bank.
  - Segment softmax over atoms:   Wc = exp(max_l S); Sc = 1^T (ind * Wc).
  - Residue softmax over protein: Wp = max_a S via PE transpose + grouped
    reduce; ew = exp(Wp); t = sum_l ew via ones-matmul.
  - Pools via matmuls (atoms / residues on the contraction partitions),
    normalization via a k=1 broadcast matmul + reciprocal, then the 3-layer
    MLP entirely on-chip per core ([256]->512->256->1 for its 8 molecules).

All shapes are static and identical across cores (single SPMD program);
per-core differences (counts, indicators, padding) live in the DMA'd data.
"""

import numpy as np

import concourse.bacc as bacc
import concourse.bass as bass
import concourse.tile as tile
from concourse import mybir
from concourse.bass_utils import run_bass_kernel_spmd

F32 = mybir.dt.float32
F16 = mybir.dt.float16
AxX = mybir.AxisListType.X
AF = mybir.ActivationFunctionType

A, L, D, B = 2048, 512, 128, 64
H1, H2 = 512, 256
NCORES = 8
MPC = B // NCORES            # molecules per core = 8
NPAD = 64                    # padded atom slots per molecule
NSTACK = MPC * NPAD // 128   # stacks of 128 padded atoms per core = 4

# fp16 consts tensor column layout
C_IDENT = 0        # [0, 128)   identity
C_IND = 128        # [128, 136) indicator, col = molecule
C_ONES = 136       # [136, 137) ones column
C_WO = 137         # [137, 139) Wo chunks
C_W = 139

_PROGRAM_CACHE = {}


def _build_program():
    nc = bacc.Bacc("TRN2", target_bir_lowering=False, debug=False)

    d_protT = nc.dram_tensor("protT", [128, MPC * L], F16, kind="ExternalInput")
    d_pnat = nc.dram_tensor("pnat", [128, MPC * L], F16, kind="ExternalInput")
    d_atomT = nc.dram_tensor("atomT", [128, MPC * NPAD], F16, kind="ExternalInput")
    d_atomN = nc.dram_tensor("atomN", [128, NSTACK * D], F16, kind="ExternalInput")
    d_watt = nc.dram_tensor("watt", [128, D], F16, kind="ExternalInput")
    d_w1 = nc.dram_tensor("w1", [128, 2 * H1], F16, kind="ExternalInput")
    d_w2 = nc.dram_tensor("w2", [128, 4 * H2], F16, kind="ExternalInput")
    d_consts = nc.dram_tensor("consts", [128, C_W], F16, kind="ExternalInput")
    d_bias = nc.dram_tensor("biasc", [128, 6], F32, kind="ExternalInput")
    d_row = nc.dram_tensor("row", [1, 129], F16, kind="ExternalInput")
    d_y = nc.dram_tensor("y", [MPC, 1], F32, kind="ExternalOutput")

    with tile.TileContext(nc) as tc:
        with (
            tc.tile_pool(name="weights", bufs=1) as wpool,
            tc.tile_pool(name="work", bufs=1) as work,
            tc.tile_pool(name="spool", bufs=2) as spool,
            tc.tile_pool(name="psum_big", bufs=4, space=bass.MemorySpace.PSUM) as pbig,
            tc.tile_pool(name="psum_q", bufs=2, space=bass.MemorySpace.PSUM) as pq,
            tc.tile_pool(name="psum_s", bufs=2, space=bass.MemorySpace.PSUM) as ps,
        ):
            # ---- loads -------------------------------------------------
            protT = wpool.tile([128, MPC * L], F16)
            nc.sync.dma_start(protT[:], d_protT[:])
            pnat = wpool.tile([128, MPC * L], F16)
            nc.sync.dma_start(pnat[:], d_pnat[:])
            atomT = wpool.tile([128, MPC * NPAD], F16)
            nc.sync.dma_start(atomT[:], d_atomT[:])
            atomN = wpool.tile([128, NSTACK, D], F16)
            nc.sync.dma_start(atomN[:], d_atomN[:])
            watt = wpool.tile([128, D], F16)
            nc.sync.dma_start(watt[:], d_watt[:])
            w1 = wpool.tile([128, 2 * H1], F16)
            nc.sync.dma_start(w1[:], d_w1[:])
            w2 = wpool.tile([128, 4 * H2], F16)
            nc.sync.dma_start(w2[:], d_w2[:])
            consts = wpool.tile([128, C_W], F16)
            nc.sync.dma_start(consts[:], d_consts[:])
            biasc = wpool.tile([128, 6], F32)
            nc.sync.dma_start(biasc[:], d_bias[:])
            row = wpool.tile([1, 129], F16)
            nc.sync.dma_start(row[:], d_row[:])

            ident = consts[:, C_IDENT : C_IDENT + 128]
            ones_col = consts[:, C_ONES : C_ONES + 1]

            # ---- XT = W_att.T-applied atoms: XT[d', a] -----------------
            ps_xt = pbig.tile([128, MPC * NPAD], F32, tag="big")
            nc.tensor.matmul(ps_xt[:], watt[:], atomT[:], start=True, stop=True)
            xt = work.tile([128, MPC * NPAD], F16)
            nc.vector.tensor_copy(xt[:], ps_xt[:])

            # ---- scores: S[a, l] per molecule, stacked 2/psum bank -----
            s_psums = []
            for s in range(NSTACK):
                ps_S = pbig.tile([128, L], F32, tag="big")
                s_psums.append(ps_S)
                for slot in range(2):
                    i = 2 * s + slot
                    nc.tensor.matmul(
                        ps_S[slot * NPAD : (slot + 1) * NPAD, :],
                        xt[:, i * NPAD : (i + 1) * NPAD],
                        protT[:, i * L : (i + 1) * L],
                        start=True,
                        stop=True,
                    )

            # ---- S to SBUF; Wc on gpsimd; transposes; Wp ---------------
            wc = work.tile([128, NSTACK], F32)
            wp = work.tile([128, 4 * MPC], F32)  # col = 8*j + m
            for s in range(NSTACK):
                s_sb = spool.tile([128, L], F16, tag="s_sb")
                if s % 2 == 0:
                    nc.scalar.copy(s_sb[:], s_psums[s][:])
                else:
                    nc.vector.tensor_copy(s_sb[:], s_psums[s][:])
                nc.vector.reduce_max(wc[:, s : s + 1], s_psums[s][:], axis=AxX)
                for j in range(4):
                    ps_st = pq.tile([128, 128], F16, tag="q")
                    nc.tensor.transpose(
                        ps_st[:], s_sb[:, j * 128 : (j + 1) * 128], ident
                    )
                    nc.vector.reduce_max(
                        wp[:, 8 * j + 2 * s : 8 * j + 2 * s + 2],
                        ps_st[:].rearrange("p (g k) -> p g k", k=NPAD),
                        axis=AxX,
                    )
            wce = work.tile([128, NSTACK], F32)
            nc.scalar.activation(wce[:], wc[:], AF.Exp)
            ew = work.tile([128, 4 * MPC], F16)
            nc.scalar.activation(ew[:], wp[:], AF.Exp)

            # ---- denominators: Sc and t --------------------------------
            wcseg = work.tile([128, MPC], F16)
            for s in range(NSTACK):
                nc.vector.tensor_scalar_mul(
                    wcseg[:, 2 * s : 2 * s + 2],
                    in0=consts[:, C_IND + 2 * s : C_IND + 2 * s + 2],
                    scalar1=wce[:, s : s + 1],
                )
            ps_sc = ps.tile([1, MPC], F32, tag="sp")
            nc.tensor.matmul(ps_sc[:], ones_col, wcseg[:], start=True, stop=True)

            tpart = work.tile([128, MPC], F16)
            with nc.allow_low_precision(reason="sum of 4 fp16 values, 5e-4 rel"):
                nc.vector.reduce_sum(
                    tpart[:], ew[:].rearrange("p (j m) -> p m j", m=MPC), axis=AxX
                )
            ps_t = ps.tile([1, MPC], F32, tag="sp")
            nc.tensor.matmul(ps_t[:], ones_col, tpart[:], start=True, stop=True)

            sct = work.tile([1, 2 * MPC], F16)
            nc.vector.tensor_copy(sct[:, :MPC], ps_sc[:])
            nc.vector.tensor_copy(sct[:, MPC:], ps_t[:])
            ps_bc = ps.tile([128, 2 * MPC], F32, tag="sp")
            nc.tensor.matmul(ps_bc[:], row[:, :128], sct[:], start=True, stop=True)
            inv = work.tile([128, 2 * MPC], F32)
            nc.vector.reciprocal(inv[:], ps_bc[:])

            # ---- pools -------------------------------------------------
            ps_ap = ps.tile([128, MPC], F32, tag="sp")
            for s in range(NSTACK):
                nc.tensor.matmul(
                    ps_ap[:, 2 * s : 2 * s + 2],
                    atomN[:, s, :],
                    wcseg[:, 2 * s : 2 * s + 2],
                    start=True,
                    stop=True,
                )
            ps_pp = ps.tile([128, MPC], F32, tag="sp")
            for m in range(MPC):
                for j in range(4):
                    nc.tensor.matmul(
                        ps_pp[:, m : m + 1],
                        pnat[:, (4 * m + j) * 128 : (4 * m + j + 1) * 128],
                        ew[:, 8 * j + m : 8 * j + m + 1],
                        start=(j == 0),
                        stop=(j == 3),
                    )

            htop = work.tile([128, MPC], F16)
            nc.vector.tensor_mul(htop[:], ps_ap[:], inv[:, :MPC])
            hbot = work.tile([128, MPC], F16)
            nc.vector.tensor_mul(hbot[:], ps_pp[:], inv[:, MPC:])

            # ---- MLP ---------------------------------------------------
            h1 = work.tile([128, 4 * MPC], F16)
            for mc in range(4):
                ps_h1 = ps.tile([128, MPC], F32, tag="sp")
                nc.tensor.matmul(
                    ps_h1[:],
                    w1[:, mc * 128 : (mc + 1) * 128],
                    htop[:],
                    start=True,
                    stop=False,
                )
                nc.tensor.matmul(
                    ps_h1[:],
                    w1[:, H1 + mc * 128 : H1 + (mc + 1) * 128],
                    hbot[:],
                    start=False,
                    stop=True,
                )
                nc.scalar.activation(
                    h1[:, mc * MPC : (mc + 1) * MPC],
                    ps_h1[:],
                    AF.Relu,
                    bias=biasc[:, mc : mc + 1],
                )
            h2 = work.tile([128, 2 * MPC], F16)
            for mc2 in range(2):
                ps_h2 = ps.tile([128, MPC], F32, tag="sp")
                for kc in range(4):
                    nc.tensor.matmul(
                        ps_h2[:],
                        w2[:, kc * H2 + mc2 * 128 : kc * H2 + (mc2 + 1) * 128],
                        h1[:, kc * MPC : (kc + 1) * MPC],
                        start=(kc == 0),
                        stop=(kc == 3),
                    )
                nc.scalar.activation(
                    h2[:, mc2 * MPC : (mc2 + 1) * MPC],
                    ps_h2[:],
                    AF.Relu,
                    bias=biasc[:, 4 + mc2 : 4 + mc2 + 1],
                )
            ps_o = ps.tile([MPC, 1], F32, tag="sp")
            nc.tensor.matmul(
                ps_o[:], h2[:, :MPC], consts[:, C_WO : C_WO + 1], start=True, stop=False
            )
            nc.tensor.matmul(
                ps_o[:],
                h2[:, MPC : 2 * MPC],
                consts[:, C_WO + 1 : C_WO + 2],
                start=False,
                stop=False,
            )
            nc.tensor.matmul(
                ps_o[:], row[:, :MPC], row[:, 128:129], start=False, stop=True
            )
            y_sb = work.tile([MPC, 1], F32)
            nc.vector.tensor_copy(y_sb[:], ps_o[:])
            nc.sync.dma_start(d_y[:], y_sb[:])

    nc.compile()
    return nc


def _prep_inputs(atom_embed, protSeq_embed, atom_splits, W_att, W1, b1, W2, b2, Wo, bo):
    f16 = np.float16
    atom = np.asarray(atom_embed, dtype=np.float32)
    prot = np.asarray(protSeq_embed, dtype=np.float32)
    splits = np.asarray(atom_splits).astype(np.int64).ravel()
    order = np.argsort(splits, kind="stable")
    counts = np.bincount(splits, minlength=B)
    assert counts.max() <= NPAD, f"molecule with {counts.max()} atoms > NPAD={NPAD}"
    assert counts.min() >= 1, "empty molecule (reference produces NaN there)"
    offs = np.concatenate([[0], np.cumsum(counts)])

    atomP = np.empty((B, NPAD, D), np.float32)
    ind = np.zeros((B, NPAD), np.float32)
    for b in range(B):
        idx = order[offs[b] : offs[b + 1]]
        n = len(idx)
        atomP[b, :n] = atom[idx]
        atomP[b, n:] = atom[idx[0]]  # replicate a real atom: maxes stay exact
        ind[b, :n] = 1.0

    w_att = np.asarray(W_att, np.float32).astype(f16)
    w1h = (
        np.asarray(W1, np.float32)
        .reshape(2, 128, H1).transpose(1, 0, 2).reshape(128, 2 * H1).astype(f16)
    )
    w2h = (
        np.asarray(W2, np.float32)
        .reshape(4, 128, H2).transpose(1, 0, 2).reshape(128, 4 * H2).astype(f16)
    )
    b1c = np.asarray(b1, np.float32).reshape(4, 128).T
    b2c = np.asarray(b2, np.float32).reshape(2, 128).T
    biasc = np.zeros((128, 6), np.float32)
    biasc[:, 0:4] = b1c
    biasc[:, 4:6] = b2c
    woc = np.asarray(Wo, np.float32).reshape(2, 128).T.astype(f16)
    row = np.zeros((1, 129), f16)
    row[0, :128] = 1.0
    row[0, 128] = np.asarray(bo, np.float32).ravel()[0]

    in_maps = []
    for c in range(NCORES):
        sl = slice(c * MPC, (c + 1) * MPC)
        protT_c = np.ascontiguousarray(
            prot[sl].transpose(2, 0, 1).reshape(128, MPC * L).astype(f16)
        )
        pnat_c = np.ascontiguousarray(
            prot[sl].reshape(MPC, 4, 128, D).transpose(2, 0, 1, 3)
            .reshape(128, MPC * L).astype(f16)
        )
        atomT_c = np.ascontiguousarray(atomP[sl].reshape(MPC * NPAD, D).T.astype(f16))
        atomN_c = np.ascontiguousarray(
            atomP[sl].reshape(NSTACK, 128, D).transpose(1, 0, 2)
            .reshape(128, NSTACK * D).astype(f16)
        )
        ind_c = np.zeros((128, MPC), f16)
        for m in range(MPC):
            s, slot = divmod(m, 2)
            ind_c[slot * NPAD : (slot + 1) * NPAD, m] = ind[c * MPC + m]
        consts = np.zeros((128, C_W), f16)
        consts[:, C_IDENT : C_IDENT + 128] = np.eye(128, dtype=f16)
        consts[:, C_IND : C_IND + MPC] = ind_c
        consts[:, C_ONES] = 1.0
        consts[:, C_WO : C_WO + 2] = woc
        in_maps.append(
            {
                "protT": protT_c,
                "pnat": pnat_c,
                "atomT": atomT_c,
                "atomN": atomN_c,
                "watt": w_att,
                "w1": w1h,
                "w2": w2h,
                "consts": consts,
                "biasc": biasc,
                "row": row,
            }
        )
    return in_maps


def kernel(atom_embed, protSeq_embed, atom_splits, W_att, W1, b1, W2, b2, Wo, bo,
           _trace=False):
    if "nc" not in _PROGRAM_CACHE:
        _PROGRAM_CACHE["nc"] = _build_program()
    nc = _PROGRAM_CACHE["nc"]
    in_maps = _prep_inputs(
        atom_embed, protSeq_embed, atom_splits, W_att, W1, b1, W2, b2, Wo, bo
    )
    res = run_bass_kernel_spmd(
        nc, in_maps, core_ids=list(range(NCORES)), trace=_trace
    )
    _PROGRAM_CACHE["last_result"] = res
    out = np.concatenate([res.results[c]["y"] for c in range(NCORES)], axis=0)
    return out.astype(np.float32)


# revision 15
# speedup vs baseline: 2.5027x; 1.1714x over previous
"""Trainium2 Bass kernel for nn_BiInteraction (segment softmax bi-interaction).

Strategy (data-parallel over molecules, 8 NeuronCores):
  - Each core owns 8 molecules (its contiguous slice of the batch) and gets
    its slice of protSeq_embed in two layouts (host-transposed protT for the
    score matmuls; natural-layout chunks for the attention pool), its atoms
    padded to 64 slots per molecule (pads are REPLICAS of a real atom, which
    keeps every max reduction exact without masks), an indicator matrix for
    segment sums, and the replicated MLP weights.
  - All matmul operands are fp16 (PSUM accumulation stays fp32): single-pass
    PE matmuls (fp32 runs LOW_HIGH double-pass at 4x the cost) and half the
    HBM traffic. End-to-end error vs the fp32 reference is ~1e-3.
  - Scores S[a, l] = (atom @ W_att) . prot[seg(a), l] are computed
    block-diagonally: one matmul per molecule (k = d = 128 contraction),
    two molecules stacked per PSUM bank.
  - Segment softmax over atoms:   Wc = exp(max_l S); Sc = 1^T (ind * Wc).
  - Residue softmax over protein: Wp = max_a S via PE transpose + grouped
    reduce; ew = exp(Wp); t = sum_l ew via ones-matmul.
  - Pools via matmuls (atoms / residues on the contraction partitions),
    normalization via a k=1 broadcast matmul + reciprocal, then the 3-layer
    MLP entirely on-chip per core ([256]->512->256->1 for its 8 molecules).

All shapes are static and identical across cores (single SPMD program);
per-core differences (counts, indicators, padding) live in the DMA'd data.
"""

import numpy as np

import concourse.bacc as bacc
import concourse.bass as bass
import concourse.tile as tile
from concourse import mybir
from concourse.bass_utils import run_bass_kernel_spmd

F32 = mybir.dt.float32
F16 = mybir.dt.float16
AxX = mybir.AxisListType.X
AF = mybir.ActivationFunctionType

A, L, D, B = 2048, 512, 128, 64
H1, H2 = 512, 256
NCORES = 8
MPC = B // NCORES            # molecules per core = 8
NPAD = 64                    # padded atom slots per molecule
NSTACK = MPC * NPAD // 128   # stacks of 128 padded atoms per core = 4

# fp16 consts tensor column layout
C_IDENT = 0        # [0, 128)   identity
C_IND = 128        # [128, 136) indicator, col = molecule
C_ONES = 136       # [136, 137) ones column
C_WO = 137         # [137, 139) Wo chunks
C_W = 139

_PROGRAM_CACHE = {}


def _build_program():
    nc = bacc.Bacc("TRN2", target_bir_lowering=False, debug=False)

    d_protT = [
        nc.dram_tensor(f"protT{m}", [128, L], F16, kind="ExternalInput")
        for m in range(MPC)
    ]
    d_pnat = [
        nc.dram_tensor(f"pnat{m}", [128, L], F16, kind="ExternalInput")
        for m in range(MPC)
    ]
    d_atomT = nc.dram_tensor("atomT", [128, MPC * NPAD], F16, kind="ExternalInput")
    d_atomN = nc.dram_tensor("atomN", [128, NSTACK * D], F16, kind="ExternalInput")
    d_watt = nc.dram_tensor("watt", [128, D], F16, kind="ExternalInput")
    d_w1 = nc.dram_tensor("w1", [128, 2 * H1], F16, kind="ExternalInput")
    d_w2 = nc.dram_tensor("w2", [128, 4 * H2], F16, kind="ExternalInput")
    d_consts = nc.dram_tensor("consts", [128, C_W], F16, kind="ExternalInput")
    d_bias = nc.dram_tensor("biasc", [128, 6], F32, kind="ExternalInput")
    d_row = nc.dram_tensor("row", [1, 129], F16, kind="ExternalInput")
    d_y = nc.dram_tensor("y", [MPC, 1], F32, kind="ExternalOutput")

    with tile.TileContext(nc) as tc:
        with (
            tc.tile_pool(name="weights", bufs=1) as wpool,
            tc.tile_pool(name="work", bufs=1) as work,
            tc.tile_pool(name="spool", bufs=2) as spool,
            tc.tile_pool(name="psum_big", bufs=4, space=bass.MemorySpace.PSUM) as pbig,
            tc.tile_pool(name="psum_q", bufs=2, space=bass.MemorySpace.PSUM) as pq,
            tc.tile_pool(name="psum_s", bufs=2, space=bass.MemorySpace.PSUM) as ps,
        ):
            # ---- loads (small/early-needed first, MLP weights last) ----
            atomT = wpool.tile([128, MPC * NPAD], F16)
            nc.sync.dma_start(atomT[:], d_atomT[:])
            watt = wpool.tile([128, D], F16)
            nc.sync.dma_start(watt[:], d_watt[:])
            consts = wpool.tile([128, C_W], F16)
            nc.sync.dma_start(consts[:], d_consts[:])
            protT = []
            for m in range(MPC):
                pt = wpool.tile([128, L], F16, tag=f"protT{m}")
                nc.sync.dma_start(pt[:], d_protT[m][:])
                protT.append(pt)
            atomN = wpool.tile([128, NSTACK, D], F16)
            nc.sync.dma_start(atomN[:], d_atomN[:])
            pnat = []
            for m in range(MPC):
                pn = wpool.tile([128, L], F16, tag=f"pnat{m}")
                nc.sync.dma_start(pn[:], d_pnat[m][:])
                pnat.append(pn)
            biasc = wpool.tile([128, 6], F32)
            nc.sync.dma_start(biasc[:], d_bias[:])
            row = wpool.tile([1, 129], F16)
            nc.sync.dma_start(row[:], d_row[:])
            w1 = wpool.tile([128, 2 * H1], F16)
            nc.sync.dma_start(w1[:], d_w1[:])
            w2 = wpool.tile([128, 4 * H2], F16)
            nc.sync.dma_start(w2[:], d_w2[:])

            ident = consts[:, C_IDENT : C_IDENT + 128]
            ones_col = consts[:, C_ONES : C_ONES + 1]

            # ---- XT = W_att.T-applied atoms: XT[d', a] -----------------
            ps_xt = pbig.tile([128, MPC * NPAD], F32, tag="big")
            nc.tensor.matmul(ps_xt[:], watt[:], atomT[:], start=True, stop=True)
            xt = work.tile([128, MPC * NPAD], F16)
            nc.vector.tensor_copy(xt[:], ps_xt[:])

            # ---- scores: S[a, l] per molecule, stacked 2/psum bank -----
            s_psums = []
            for s in range(NSTACK):
                ps_S = pbig.tile([128, L], F32, tag="big")
                s_psums.append(ps_S)
                for slot in range(2):
                    i = 2 * s + slot
                    nc.tensor.matmul(
                        ps_S[slot * NPAD : (slot + 1) * NPAD, :],
                        xt[:, i * NPAD : (i + 1) * NPAD],
                        protT[i][:],
                        start=True,
                        stop=True,
                    )

            # ---- S to SBUF; Wc; transposes; Wp -------------------------
            # wp col layout: 8*s + 2*j + slot  (molecule m = 2*s + slot)
            wc = work.tile([128, NSTACK], F32)
            wp = work.tile([128, 4 * MPC], F32)
            for s in range(NSTACK):
                s_sb = spool.tile([128, L], F16, tag="s_sb")
                if s % 2 == 0:
                    nc.scalar.copy(s_sb[:], s_psums[s][:])
                else:
                    nc.vector.tensor_copy(s_sb[:], s_psums[s][:])
                nc.vector.reduce_max(wc[:, s : s + 1], s_psums[s][:], axis=AxX)
                ps_st = pq.tile([128, 4 * 128], F16, tag="q")
                for j in range(4):
                    nc.tensor.transpose(
                        ps_st[:, j * 128 : (j + 1) * 128],
                        s_sb[:, j * 128 : (j + 1) * 128],
                        ident,
                    )
                nc.vector.reduce_max(
                    wp[:, 8 * s : 8 * (s + 1)],
                    ps_st[:].rearrange("p (j g k) -> p j g k", j=4, k=NPAD),
                    axis=AxX,
                )
            wce = work.tile([128, NSTACK], F32)
            nc.scalar.activation(wce[:], wc[:], AF.Exp)
            ew = work.tile([128, 4 * MPC], F16)
            nc.scalar.activation(ew[:], wp[:], AF.Exp)

            # ---- denominators: Sc and t --------------------------------
            wcseg = work.tile([128, MPC], F16)
            for s in range(NSTACK):
                nc.vector.tensor_scalar_mul(
                    wcseg[:, 2 * s : 2 * s + 2],
                    in0=consts[:, C_IND + 2 * s : C_IND + 2 * s + 2],
                    scalar1=wce[:, s : s + 1],
                )
            ps_sc = ps.tile([1, MPC], F32, tag="sp")
            nc.tensor.matmul(ps_sc[:], ones_col, wcseg[:], start=True, stop=True)

            tpart = work.tile([128, MPC], F16)
            with nc.allow_low_precision(reason="sum of 4 fp16 values, 5e-4 rel"):
                nc.vector.reduce_sum(
                    tpart[:].rearrange("p (s sl) -> p s sl", sl=2),
                    ew[:].rearrange("p (s j sl) -> p s sl j", j=4, sl=2),
                    axis=AxX,
                )
            ps_t = ps.tile([1, MPC], F32, tag="sp")
            nc.tensor.matmul(ps_t[:], ones_col, tpart[:], start=True, stop=True)

            sct = work.tile([1, 2 * MPC], F16)
            nc.vector.tensor_copy(sct[:, :MPC], ps_sc[:])
            nc.vector.tensor_copy(sct[:, MPC:], ps_t[:])
            ps_bc = ps.tile([128, 2 * MPC], F32, tag="sp")
            nc.tensor.matmul(ps_bc[:], row[:, :128], sct[:], start=True, stop=True)
            inv = work.tile([128, 2 * MPC], F32)
            nc.vector.reciprocal(inv[:], ps_bc[:])

            # ---- pools -------------------------------------------------
            ps_ap = ps.tile([128, MPC], F32, tag="sp")
            for s in range(NSTACK):
                nc.tensor.matmul(
                    ps_ap[:, 2 * s : 2 * s + 2],
                    atomN[:, s, :],
                    wcseg[:, 2 * s : 2 * s + 2],
                    start=True,
                    stop=True,
                )
            # row-form pools packed 4 per PE column-group: molecule g*4+sl
            # accumulates in row 32*sl of psum tile prow[g].
            prows = []
            for g in range(2):
                ps_pr = pq.tile([128, 128], F32, tag="q")
                prows.append(ps_pr)
                for j in range(4):
                    for sl in range(4):
                        m = 4 * g + sl
                        ewc = 8 * (m // 2) + 2 * j + (m % 2)
                        nc.tensor.matmul(
                            ps_pr[32 * sl : 32 * sl + 1, :],
                            ew[:, ewc : ewc + 1],
                            pnat[m][:, j * 128 : (j + 1) * 128],
                            start=(j == 0),
                            stop=(j == 3),
                            tile_position=(0, 32 * sl),
                        )
            ps_ppT = []
            for g in range(2):
                pr_sb = work.tile([128, 128], F16, tag=f"prsb{g}")
                nc.scalar.copy(pr_sb[:], prows[g][:])
                ps_pt = pq.tile([128, 128], F16, tag="q")
                nc.tensor.transpose(ps_pt[:], pr_sb[:], ident)
                ps_ppT.append(ps_pt)

            htop = work.tile([128, MPC], F16)
            nc.vector.tensor_mul(htop[:], ps_ap[:], inv[:, :MPC])
            hbot = work.tile([128, MPC], F16)
            for g in range(2):
                nc.vector.tensor_mul(
                    hbot[:, 4 * g : 4 * g + 4],
                    ps_ppT[g][:].rearrange("p (a b) -> p b a", b=32)[:, 0, :],
                    inv[:, MPC + 4 * g : MPC + 4 * g + 4],
                )

            # ---- MLP ---------------------------------------------------
            h1 = work.tile([128, 4 * MPC], F16)
            for mc in range(4):
                ps_h1 = ps.tile([128, MPC], F32, tag="sp")
                nc.tensor.matmul(
                    ps_h1[:],
                    w1[:, mc * 128 : (mc + 1) * 128],
                    htop[:],
                    start=True,
                    stop=False,
                )
                nc.tensor.matmul(
                    ps_h1[:],
                    w1[:, H1 + mc * 128 : H1 + (mc + 1) * 128],
                    hbot[:],
                    start=False,
                    stop=True,
                )
                nc.scalar.activation(
                    h1[:, mc * MPC : (mc + 1) * MPC],
                    ps_h1[:],
                    AF.Relu,
                    bias=biasc[:, mc : mc + 1],
                )
            h2 = work.tile([128, 2 * MPC], F16)
            for mc2 in range(2):
                ps_h2 = ps.tile([128, MPC], F32, tag="sp")
                for kc in range(4):
                    nc.tensor.matmul(
                        ps_h2[:],
                        w2[:, kc * H2 + mc2 * 128 : kc * H2 + (mc2 + 1) * 128],
                        h1[:, kc * MPC : (kc + 1) * MPC],
                        start=(kc == 0),
                        stop=(kc == 3),
                    )
                nc.scalar.activation(
                    h2[:, mc2 * MPC : (mc2 + 1) * MPC],
                    ps_h2[:],
                    AF.Relu,
                    bias=biasc[:, 4 + mc2 : 4 + mc2 + 1],
                )
            ps_o = ps.tile([MPC, 1], F32, tag="sp")
            nc.tensor.matmul(
                ps_o[:], h2[:, :MPC], consts[:, C_WO : C_WO + 1], start=True, stop=False
            )
            nc.tensor.matmul(
                ps_o[:],
                h2[:, MPC : 2 * MPC],
                consts[:, C_WO + 1 : C_WO + 2],
                start=False,
                stop=False,
            )
            nc.tensor.matmul(
                ps_o[:], row[:, :MPC], row[:, 128:129], start=False, stop=True
            )
            y_sb = work.tile([MPC, 1], F32)
            nc.vector.tensor_copy(y_sb[:], ps_o[:])
            nc.sync.dma_start(d_y[:], y_sb[:])

    nc.compile()
    return nc


def _prep_inputs(atom_embed, protSeq_embed, atom_splits, W_att, W1, b1, W2, b2, Wo, bo):
    f16 = np.float16
    atom = np.asarray(atom_embed, dtype=np.float32)
    prot = np.asarray(protSeq_embed, dtype=np.float32)
    splits = np.asarray(atom_splits).astype(np.int64).ravel()
    order = np.argsort(splits, kind="stable")
    counts = np.bincount(splits, minlength=B)
    assert counts.max() <= NPAD, f"molecule with {counts.max()} atoms > NPAD={NPAD}"
    assert counts.min() >= 1, "empty molecule (reference produces NaN there)"
    offs = np.concatenate([[0], np.cumsum(counts)])

    atomP = np.empty((B, NPAD, D), np.float32)
    ind = np.zeros((B, NPAD), np.float32)
    for b in range(B):
        idx = order[offs[b] : offs[b + 1]]
        n = len(idx)
        atomP[b, :n] = atom[idx]
        atomP[b, n:] = atom[idx[0]]  # replicate a real atom: maxes stay exact
        ind[b, :n] = 1.0

    w_att = np.asarray(W_att, np.float32).astype(f16)
    w1h = (
        np.asarray(W1, np.float32)
        .reshape(2, 128, H1).transpose(1, 0, 2).reshape(128, 2 * H1).astype(f16)
    )
    w2h = (
        np.asarray(W2, np.float32)
        .reshape(4, 128, H2).transpose(1, 0, 2).reshape(128, 4 * H2).astype(f16)
    )
    b1c = np.asarray(b1, np.float32).reshape(4, 128).T
    b2c = np.asarray(b2, np.float32).reshape(2, 128).T
    biasc = np.zeros((128, 6), np.float32)
    biasc[:, 0:4] = b1c
    biasc[:, 4:6] = b2c
    woc = np.asarray(Wo, np.float32).reshape(2, 128).T.astype(f16)
    row = np.zeros((1, 129), f16)
    row[0, :128] = 1.0
    row[0, 128] = np.asarray(bo, np.float32).ravel()[0]

    in_maps = []
    for c in range(NCORES):
        sl = slice(c * MPC, (c + 1) * MPC)
        protT_c = np.ascontiguousarray(
            prot[sl].transpose(0, 2, 1).astype(f16)
        )  # [MPC, 128, L]
        pnat_c = np.ascontiguousarray(
            prot[sl].reshape(MPC, 4, 128, D).transpose(0, 2, 1, 3)
            .reshape(MPC, 128, L).astype(f16)
        )
        atomT_c = np.ascontiguousarray(atomP[sl].reshape(MPC * NPAD, D).T.astype(f16))
        atomN_c = np.ascontiguousarray(
            atomP[sl].reshape(NSTACK, 128, D).transpose(1, 0, 2)
            .reshape(128, NSTACK * D).astype(f16)
        )
        ind_c = np.zeros((128, MPC), f16)
        for m in range(MPC):
            s, slot = divmod(m, 2)
            ind_c[slot * NPAD : (slot + 1) * NPAD, m] = ind[c * MPC + m]
        consts = np.zeros((128, C_W), f16)
        consts[:, C_IDENT : C_IDENT + 128] = np.eye(128, dtype=f16)
        consts[:, C_IND : C_IND + MPC] = ind_c
        consts[:, C_ONES] = 1.0
        consts[:, C_WO : C_WO + 2] = woc
        im = {
                "atomT": atomT_c,
                "atomN": atomN_c,
                "watt": w_att,
                "w1": w1h,
                "w2": w2h,
                "consts": consts,
                "biasc": biasc,
                "row": row,
        }
        for m in range(MPC):
            im[f"protT{m}"] = protT_c[m]
            im[f"pnat{m}"] = pnat_c[m]
        in_maps.append(im)
    return in_maps


def kernel(atom_embed, protSeq_embed, atom_splits, W_att, W1, b1, W2, b2, Wo, bo,
           _trace=False):
    if "nc" not in _PROGRAM_CACHE:
        _PROGRAM_CACHE["nc"] = _build_program()
    nc = _PROGRAM_CACHE["nc"]
    in_maps = _prep_inputs(
        atom_embed, protSeq_embed, atom_splits, W_att, W1, b1, W2, b2, Wo, bo
    )
    res = run_bass_kernel_spmd(
        nc, in_maps, core_ids=list(range(NCORES)), trace=_trace
    )
    _PROGRAM_CACHE["last_result"] = res
    out = np.concatenate([res.results[c]["y"] for c in range(NCORES)], axis=0)
    return out.astype(np.float32)


# revision 16
# speedup vs baseline: 2.5142x; 1.0046x over previous
"""Trainium2 Bass kernel for nn_BiInteraction (segment softmax bi-interaction).

Strategy (data-parallel over molecules, 8 NeuronCores):
  - Each core owns 8 molecules (its contiguous slice of the batch) and gets
    its slice of protSeq_embed in two layouts (host-transposed protT for the
    score matmuls; natural-layout chunks for the attention pool), its atoms
    padded to 64 slots per molecule (pads are REPLICAS of a real atom, which
    keeps every max reduction exact without masks), an indicator matrix for
    segment sums, and the replicated MLP weights.
  - All matmul operands are fp16 (PSUM accumulation stays fp32): single-pass
    PE matmuls (fp32 runs LOW_HIGH double-pass at 4x the cost) and half the
    HBM traffic. End-to-end error vs the fp32 reference is ~1e-3.
  - Scores S[a, l] = (atom @ W_att) . prot[seg(a), l] are computed
    block-diagonally: one matmul per molecule (k = d = 128 contraction),
    two molecules stacked per PSUM bank.
  - Segment softmax over atoms:   Wc = exp(max_l S); Sc = 1^T (ind * Wc).
  - Residue softmax over protein: Wp = max_a S via PE transpose + grouped
    reduce; ew = exp(Wp); t = sum_l ew via ones-matmul.
  - Pools via matmuls (atoms / residues on the contraction partitions),
    normalization via a k=1 broadcast matmul + reciprocal, then the 3-layer
    MLP entirely on-chip per core ([256]->512->256->1 for its 8 molecules).

All shapes are static and identical across cores (single SPMD program);
per-core differences (counts, indicators, padding) live in the DMA'd data.
"""

import numpy as np

import concourse.bacc as bacc
import concourse.bass as bass
import concourse.tile as tile
from concourse import mybir
from concourse.bass_utils import run_bass_kernel_spmd

F32 = mybir.dt.float32
F16 = mybir.dt.float16
AxX = mybir.AxisListType.X
AF = mybir.ActivationFunctionType

A, L, D, B = 2048, 512, 128, 64
H1, H2 = 512, 256
NCORES = 8
MPC = B // NCORES            # molecules per core = 8
NPAD = 64                    # padded atom slots per molecule
NSTACK = MPC * NPAD // 128   # stacks of 128 padded atoms per core = 4

# fp16 consts tensor column layout
C_IDENT = 0        # [0, 128)   identity
C_IND = 128        # [128, 136) indicator, col = molecule
C_ONES = 136       # [136, 137) ones column
C_WO = 137         # [137, 139) Wo chunks
C_W = 139

_PROGRAM_CACHE = {}


def _build_program():
    nc = bacc.Bacc("TRN2", target_bir_lowering=False, debug=False)

    # acp = atomT | watt | atomN | consts, one early DMA
    ACP_W = MPC * NPAD + D + NSTACK * D + C_W
    d_acp = nc.dram_tensor("acp", [128, ACP_W], F16, kind="ExternalInput")
    d_protq = [
        nc.dram_tensor(f"protq{q}", [128, 4 * L], F16, kind="ExternalInput")
        for q in range(2)
    ]
    d_pnatq = [
        nc.dram_tensor(f"pnatq{q}", [128, 4 * L], F16, kind="ExternalInput")
        for q in range(2)
    ]
    d_w12 = nc.dram_tensor("w12", [128, 2 * H1 + 4 * H2], F16, kind="ExternalInput")
    d_bias = nc.dram_tensor("biasc", [128, 6], F32, kind="ExternalInput")
    d_row = nc.dram_tensor("row", [1, 129], F16, kind="ExternalInput")
    d_y = nc.dram_tensor("y", [MPC, 1], F32, kind="ExternalOutput")

    with tile.TileContext(nc) as tc:
        with (
            tc.tile_pool(name="weights", bufs=1) as wpool,
            tc.tile_pool(name="work", bufs=1) as work,
            tc.tile_pool(name="spool", bufs=2) as spool,
            tc.tile_pool(name="psum_big", bufs=4, space=bass.MemorySpace.PSUM) as pbig,
            tc.tile_pool(name="psum_q", bufs=2, space=bass.MemorySpace.PSUM) as pq,
            tc.tile_pool(name="psum_s", bufs=2, space=bass.MemorySpace.PSUM) as ps,
        ):
            # ---- loads: acp first, prot quads, weights, misc -----------
            acp = wpool.tile([128, ACP_W], F16)
            nc.sync.dma_start(acp[:], d_acp[:])
            atomT = acp[:, 0 : MPC * NPAD]
            watt = acp[:, MPC * NPAD : MPC * NPAD + D]
            atomN = acp[:, MPC * NPAD + D : MPC * NPAD + D + NSTACK * D].rearrange(
                "p (s d) -> p s d", s=NSTACK
            )
            consts = acp[:, MPC * NPAD + D + NSTACK * D :]
            protq = []
            for q in range(2):
                pt = wpool.tile([128, 4 * L], F16, tag=f"protq{q}")
                nc.sync.dma_start(pt[:], d_protq[q][:])
                protq.append(pt)
            protT = [protq[i // 4][:, (i % 4) * L : (i % 4 + 1) * L] for i in range(MPC)]
            pnatq = []
            for q in range(2):
                pn = wpool.tile([128, 4 * L], F16, tag=f"pnatq{q}")
                nc.sync.dma_start(pn[:], d_pnatq[q][:])
                pnatq.append(pn)
            pnat = [pnatq[i // 4][:, (i % 4) * L : (i % 4 + 1) * L] for i in range(MPC)]
            w12 = wpool.tile([128, 2 * H1 + 4 * H2], F16)
            nc.sync.dma_start(w12[:], d_w12[:])
            w1 = w12[:, 0 : 2 * H1]
            w2 = w12[:, 2 * H1 :]
            biasc = wpool.tile([128, 6], F32)
            nc.sync.dma_start(biasc[:], d_bias[:])
            row = wpool.tile([1, 129], F16)
            nc.sync.dma_start(row[:], d_row[:])

            ident = consts[:, C_IDENT : C_IDENT + 128]
            ones_col = consts[:, C_ONES : C_ONES + 1]

            # ---- XT = W_att.T-applied atoms: XT[d', a] -----------------
            ps_xt = pbig.tile([128, MPC * NPAD], F32, tag="big")
            nc.tensor.matmul(ps_xt[:], watt[:], atomT[:], start=True, stop=True)
            xt = work.tile([128, MPC * NPAD], F16)
            nc.vector.tensor_copy(xt[:], ps_xt[:])

            # ---- scores: S[a, l] per molecule, stacked 2/psum bank -----
            s_psums = []
            for s in range(NSTACK):
                ps_S = pbig.tile([128, L], F32, tag="big")
                s_psums.append(ps_S)
                for slot in range(2):
                    i = 2 * s + slot
                    nc.tensor.matmul(
                        ps_S[slot * NPAD : (slot + 1) * NPAD, :],
                        xt[:, i * NPAD : (i + 1) * NPAD],
                        protT[i],
                        start=True,
                        stop=True,
                    )

            # ---- S to SBUF; Wc; transposes; Wp -------------------------
            # wp col layout: 8*s + 2*j + slot  (molecule m = 2*s + slot)
            wc = work.tile([128, NSTACK], F32)
            wp = work.tile([128, 4 * MPC], F32)
            for s in range(NSTACK):
                s_sb = spool.tile([128, L], F16, tag="s_sb")
                if s % 2 == 0:
                    nc.scalar.copy(s_sb[:], s_psums[s][:])
                else:
                    nc.vector.tensor_copy(s_sb[:], s_psums[s][:])
                nc.vector.reduce_max(wc[:, s : s + 1], s_psums[s][:], axis=AxX)
                ps_st = pq.tile([128, 4 * 128], F16, tag="q")
                for j in range(4):
                    nc.tensor.transpose(
                        ps_st[:, j * 128 : (j + 1) * 128],
                        s_sb[:, j * 128 : (j + 1) * 128],
                        ident,
                    )
                nc.vector.reduce_max(
                    wp[:, 8 * s : 8 * (s + 1)],
                    ps_st[:].rearrange("p (j g k) -> p j g k", j=4, k=NPAD),
                    axis=AxX,
                )
            wce = work.tile([128, NSTACK], F32)
            nc.scalar.activation(wce[:], wc[:], AF.Exp)
            ew = work.tile([128, 4 * MPC], F16)
            nc.scalar.activation(ew[:], wp[:], AF.Exp)

            # ---- denominators: Sc and t --------------------------------
            wcseg = work.tile([128, MPC], F16)
            for s in range(NSTACK):
                nc.vector.tensor_scalar_mul(
                    wcseg[:, 2 * s : 2 * s + 2],
                    in0=consts[:, C_IND + 2 * s : C_IND + 2 * s + 2],
                    scalar1=wce[:, s : s + 1],
                )
            ps_sc = ps.tile([1, MPC], F32, tag="sp")
            nc.tensor.matmul(ps_sc[:], ones_col, wcseg[:], start=True, stop=True)

            tpart = work.tile([128, MPC], F16)
            with nc.allow_low_precision(reason="sum of 4 fp16 values, 5e-4 rel"):
                nc.vector.reduce_sum(
                    tpart[:].rearrange("p (s sl) -> p s sl", sl=2),
                    ew[:].rearrange("p (s j sl) -> p s sl j", j=4, sl=2),
                    axis=AxX,
                )
            ps_t = ps.tile([1, MPC], F32, tag="sp")
            nc.tensor.matmul(ps_t[:], ones_col, tpart[:], start=True, stop=True)

            sct = work.tile([1, 2 * MPC], F16)
            nc.vector.tensor_copy(sct[:, :MPC], ps_sc[:])
            nc.vector.tensor_copy(sct[:, MPC:], ps_t[:])
            ps_bc = ps.tile([128, 2 * MPC], F32, tag="sp")
            nc.tensor.matmul(ps_bc[:], row[:, :128], sct[:], start=True, stop=True)
            inv = work.tile([128, 2 * MPC], F32)
            nc.vector.reciprocal(inv[:], ps_bc[:])

            # ---- pools -------------------------------------------------
            ps_ap = ps.tile([128, MPC], F32, tag="sp")
            for s in range(NSTACK):
                nc.tensor.matmul(
                    ps_ap[:, 2 * s : 2 * s + 2],
                    atomN[:, s, :],
                    wcseg[:, 2 * s : 2 * s + 2],
                    start=True,
                    stop=True,
                )
            # row-form pools packed 4 per PE column-group: molecule g*4+sl
            # accumulates in row 32*sl of psum tile prow[g].
            prows = []
            for g in range(2):
                ps_pr = pq.tile([128, 128], F32, tag="q")
                prows.append(ps_pr)
                for j in range(4):
                    for sl in range(4):
                        m = 4 * g + sl
                        ewc = 8 * (m // 2) + 2 * j + (m % 2)
                        nc.tensor.matmul(
                            ps_pr[32 * sl : 32 * sl + 1, :],
                            ew[:, ewc : ewc + 1],
                            pnat[m][:, j * 128 : (j + 1) * 128],
                            start=(j == 0),
                            stop=(j == 3),
                            tile_position=(0, 32 * sl),
                        )
            ps_ppT = []
            for g in range(2):
                pr_sb = work.tile([128, 128], F16, tag=f"prsb{g}")
                nc.scalar.copy(pr_sb[:], prows[g][:])
                ps_pt = pq.tile([128, 128], F16, tag="q")
                nc.tensor.transpose(ps_pt[:], pr_sb[:], ident)
                ps_ppT.append(ps_pt)

            htop = work.tile([128, MPC], F16)
            nc.vector.tensor_mul(htop[:], ps_ap[:], inv[:, :MPC])
            hbot = work.tile([128, MPC], F16)
            for g in range(2):
                nc.vector.tensor_mul(
                    hbot[:, 4 * g : 4 * g + 4],
                    ps_ppT[g][:].rearrange("p (a b) -> p b a", b=32)[:, 0, :],
                    inv[:, MPC + 4 * g : MPC + 4 * g + 4],
                )

            # ---- MLP ---------------------------------------------------
            h1 = work.tile([128, 4 * MPC], F16)
            for mc in range(4):
                ps_h1 = ps.tile([128, MPC], F32, tag="sp")
                nc.tensor.matmul(
                    ps_h1[:],
                    w1[:, mc * 128 : (mc + 1) * 128],
                    htop[:],
                    start=True,
                    stop=False,
                )
                nc.tensor.matmul(
                    ps_h1[:],
                    w1[:, H1 + mc * 128 : H1 + (mc + 1) * 128],
                    hbot[:],
                    start=False,
                    stop=True,
                )
                nc.scalar.activation(
                    h1[:, mc * MPC : (mc + 1) * MPC],
                    ps_h1[:],
                    AF.Relu,
                    bias=biasc[:, mc : mc + 1],
                )
            h2 = work.tile([128, 2 * MPC], F16)
            for mc2 in range(2):
                ps_h2 = ps.tile([128, MPC], F32, tag="sp")
                for kc in range(4):
                    nc.tensor.matmul(
                        ps_h2[:],
                        w2[:, kc * H2 + mc2 * 128 : kc * H2 + (mc2 + 1) * 128],
                        h1[:, kc * MPC : (kc + 1) * MPC],
                        start=(kc == 0),
                        stop=(kc == 3),
                    )
                nc.scalar.activation(
                    h2[:, mc2 * MPC : (mc2 + 1) * MPC],
                    ps_h2[:],
                    AF.Relu,
                    bias=biasc[:, 4 + mc2 : 4 + mc2 + 1],
                )
            ps_o = ps.tile([MPC, 1], F32, tag="sp")
            nc.tensor.matmul(
                ps_o[:], h2[:, :MPC], consts[:, C_WO : C_WO + 1], start=True, stop=False
            )
            nc.tensor.matmul(
                ps_o[:],
                h2[:, MPC : 2 * MPC],
                consts[:, C_WO + 1 : C_WO + 2],
                start=False,
                stop=False,
            )
            nc.tensor.matmul(
                ps_o[:], row[:, :MPC], row[:, 128:129], start=False, stop=True
            )
            y_sb = work.tile([MPC, 1], F32)
            nc.vector.tensor_copy(y_sb[:], ps_o[:])
            nc.sync.dma_start(d_y[:], y_sb[:])

    nc.compile()
    return nc


def _prep_inputs(atom_embed, protSeq_embed, atom_splits, W_att, W1, b1, W2, b2, Wo, bo):
    f16 = np.float16
    atom = np.asarray(atom_embed, dtype=np.float32)
    prot = np.asarray(protSeq_embed, dtype=np.float32)
    splits = np.asarray(atom_splits).astype(np.int64).ravel()
    order = np.argsort(splits, kind="stable")
    counts = np.bincount(splits, minlength=B)
    assert counts.max() <= NPAD, f"molecule with {counts.max()} atoms > NPAD={NPAD}"
    assert counts.min() >= 1, "empty molecule (reference produces NaN there)"
    offs = np.concatenate([[0], np.cumsum(counts)])

    atomP = np.empty((B, NPAD, D), np.float32)
    ind = np.zeros((B, NPAD), np.float32)
    for b in range(B):
        idx = order[offs[b] : offs[b + 1]]
        n = len(idx)
        atomP[b, :n] = atom[idx]
        atomP[b, n:] = atom[idx[0]]  # replicate a real atom: maxes stay exact
        ind[b, :n] = 1.0

    w_att = np.asarray(W_att, np.float32).astype(f16)  # [128, 128]
    w1h = (
        np.asarray(W1, np.float32)
        .reshape(2, 128, H1).transpose(1, 0, 2).reshape(128, 2 * H1).astype(f16)
    )
    w2h = (
        np.asarray(W2, np.float32)
        .reshape(4, 128, H2).transpose(1, 0, 2).reshape(128, 4 * H2).astype(f16)
    )
    b1c = np.asarray(b1, np.float32).reshape(4, 128).T
    b2c = np.asarray(b2, np.float32).reshape(2, 128).T
    biasc = np.zeros((128, 6), np.float32)
    biasc[:, 0:4] = b1c
    biasc[:, 4:6] = b2c
    woc = np.asarray(Wo, np.float32).reshape(2, 128).T.astype(f16)
    w12h = None  # built below
    row = np.zeros((1, 129), f16)
    row[0, :128] = 1.0
    row[0, 128] = np.asarray(bo, np.float32).ravel()[0]
    w12h = np.ascontiguousarray(np.concatenate([w1h, w2h], axis=1))

    in_maps = []
    for c in range(NCORES):
        sl = slice(c * MPC, (c + 1) * MPC)
        protT_c = np.ascontiguousarray(
            prot[sl].transpose(0, 2, 1).astype(f16)
        )  # [MPC, 128, L]
        pnat_c = np.ascontiguousarray(
            prot[sl].reshape(MPC, 4, 128, D).transpose(0, 2, 1, 3)
            .reshape(MPC, 128, L).astype(f16)
        )
        atomT_c = np.ascontiguousarray(atomP[sl].reshape(MPC * NPAD, D).T.astype(f16))
        atomN_c = np.ascontiguousarray(
            atomP[sl].reshape(NSTACK, 128, D).transpose(1, 0, 2)
            .reshape(128, NSTACK * D).astype(f16)
        )
        ind_c = np.zeros((128, MPC), f16)
        for m in range(MPC):
            s, slot = divmod(m, 2)
            ind_c[slot * NPAD : (slot + 1) * NPAD, m] = ind[c * MPC + m]
        consts = np.zeros((128, C_W), f16)
        consts[:, C_IDENT : C_IDENT + 128] = np.eye(128, dtype=f16)
        consts[:, C_IND : C_IND + MPC] = ind_c
        consts[:, C_ONES] = 1.0
        consts[:, C_WO : C_WO + 2] = woc
        acp = np.concatenate([atomT_c, w_att, atomN_c, consts], axis=1)
        im = {
            "acp": np.ascontiguousarray(acp),
            "w12": w12h,
            "biasc": biasc,
            "row": row,
        }
        for q in range(2):
            im[f"protq{q}"] = np.ascontiguousarray(
                protT_c[4 * q : 4 * q + 4].transpose(1, 0, 2).reshape(128, 4 * L)
            )
            im[f"pnatq{q}"] = np.ascontiguousarray(
                pnat_c[4 * q : 4 * q + 4].transpose(1, 0, 2).reshape(128, 4 * L)
            )
        in_maps.append(im)
    return in_maps


def kernel(atom_embed, protSeq_embed, atom_splits, W_att, W1, b1, W2, b2, Wo, bo,
           _trace=False):
    if "nc" not in _PROGRAM_CACHE:
        _PROGRAM_CACHE["nc"] = _build_program()
    nc = _PROGRAM_CACHE["nc"]
    in_maps = _prep_inputs(
        atom_embed, protSeq_embed, atom_splits, W_att, W1, b1, W2, b2, Wo, bo
    )
    res = run_bass_kernel_spmd(
        nc, in_maps, core_ids=list(range(NCORES)), trace=_trace
    )
    _PROGRAM_CACHE["last_result"] = res
    out = np.concatenate([res.results[c]["y"] for c in range(NCORES)], axis=0)
    return out.astype(np.float32)


# revision 17
# speedup vs baseline: 2.6249x; 1.0440x over previous
"""Trainium2 Bass kernel for nn_BiInteraction (segment softmax bi-interaction).

Strategy (data-parallel over molecules, 8 NeuronCores):
  - Each core owns 8 molecules (its contiguous slice of the batch) and gets
    its slice of protSeq_embed in two layouts (host-transposed protT for the
    score matmuls; natural-layout chunks for the attention pool), its atoms
    padded to 64 slots per molecule (pads are REPLICAS of a real atom, which
    keeps every max reduction exact without masks), an indicator matrix for
    segment sums, and the replicated MLP weights.
  - All matmul operands are fp16 (PSUM accumulation stays fp32): single-pass
    PE matmuls (fp32 runs LOW_HIGH double-pass at 4x the cost) and half the
    HBM traffic. End-to-end error vs the fp32 reference is ~1e-3.
  - Scores S[a, l] = (atom @ W_att) . prot[seg(a), l] are computed
    block-diagonally: one matmul per molecule (k = d = 128 contraction),
    two molecules stacked per PSUM bank.
  - Segment softmax over atoms:   Wc = exp(max_l S); Sc = 1^T (ind * Wc).
  - Residue softmax over protein: Wp = max_a S via PE transpose + grouped
    reduce; ew = exp(Wp); t = sum_l ew via ones-matmul.
  - Pools via matmuls (atoms / residues on the contraction partitions),
    normalization via a k=1 broadcast matmul + reciprocal, then the 3-layer
    MLP entirely on-chip per core ([256]->512->256->1 for its 8 molecules).

All shapes are static and identical across cores (single SPMD program);
per-core differences (counts, indicators, padding) live in the DMA'd data.
"""

import numpy as np

import concourse.bacc as bacc
import concourse.bass as bass
import concourse.tile as tile
from concourse import mybir
from concourse.bass_utils import run_bass_kernel_spmd

F32 = mybir.dt.float32
F16 = mybir.dt.float16
AxX = mybir.AxisListType.X
AF = mybir.ActivationFunctionType

A, L, D, B = 2048, 512, 128, 64
H1, H2 = 512, 256
NCORES = 8
MPC = B // NCORES            # molecules per core = 8
NPAD = 64                    # padded atom slots per molecule
NSTACK = MPC * NPAD // 128   # stacks of 128 padded atoms per core = 4

# fp16 consts tensor column layout
C_IDENT = 0        # [0, 128)   identity
C_IND = 128        # [128, 136) indicator, col = molecule
C_ONES = 136       # [136, 137) ones column
C_WO = 137         # [137, 139) Wo chunks
C_W = 139

_PROGRAM_CACHE = {}


def _build_program():
    nc = bacc.Bacc("TRN2", target_bir_lowering=False, debug=False)

    # atomw = atomT | watt (needed first); cons2 = atomN | consts
    AW_W = MPC * NPAD + D
    d_atomw = nc.dram_tensor("atomw", [128, AW_W], F16, kind="ExternalInput")
    CN_W = NSTACK * D + C_W
    d_cons2 = nc.dram_tensor("cons2", [128, CN_W], F16, kind="ExternalInput")
    d_protp = [
        nc.dram_tensor(f"protp{q}", [128, 2 * L], F16, kind="ExternalInput")
        for q in range(4)
    ]
    d_pnatq = [
        nc.dram_tensor(f"pnatq{q}", [128, 4 * L], F16, kind="ExternalInput")
        for q in range(2)
    ]
    d_w12 = nc.dram_tensor("w12", [128, 2 * H1 + 4 * H2], F16, kind="ExternalInput")
    d_bias = nc.dram_tensor("biasc", [128, 6], F32, kind="ExternalInput")
    d_row = nc.dram_tensor("row", [1, 129], F16, kind="ExternalInput")
    d_y = nc.dram_tensor("y", [MPC, 1], F32, kind="ExternalOutput")

    with tile.TileContext(nc) as tc:
        with (
            tc.tile_pool(name="weights", bufs=1) as wpool,
            tc.tile_pool(name="work", bufs=1) as work,
            tc.tile_pool(name="spool", bufs=2) as spool,
            tc.tile_pool(name="psum_big", bufs=4, space=bass.MemorySpace.PSUM) as pbig,
            tc.tile_pool(name="psum_q", bufs=2, space=bass.MemorySpace.PSUM) as pq,
            tc.tile_pool(name="psum_s", bufs=2, space=bass.MemorySpace.PSUM) as ps,
        ):
            # ---- loads: dual-queue issue, earliest-needed first --------
            atomw = wpool.tile([128, AW_W], F16)
            nc.sync.dma_start(atomw[:], d_atomw[:])
            atomT = atomw[:, 0 : MPC * NPAD]
            watt = atomw[:, MPC * NPAD :]
            cons2 = wpool.tile([128, CN_W], F16)
            nc.scalar.dma_start(cons2[:], d_cons2[:])
            atomN = cons2[:, 0 : NSTACK * D].rearrange("p (s d) -> p s d", s=NSTACK)
            consts = cons2[:, NSTACK * D :]
            protp = []
            for q in range(4):
                pt = wpool.tile([128, 2 * L], F16, tag=f"protp{q}")
                eng = nc.sync if q % 2 == 0 else nc.scalar
                eng.dma_start(pt[:], d_protp[q][:])
                protp.append(pt)
            protT = [protp[i // 2][:, (i % 2) * L : (i % 2 + 1) * L] for i in range(MPC)]
            pnatq = []
            for q in range(2):
                pn = wpool.tile([128, 4 * L], F16, tag=f"pnatq{q}")
                eng = nc.sync if q % 2 == 0 else nc.scalar
                eng.dma_start(pn[:], d_pnatq[q][:])
                pnatq.append(pn)
            pnat = [pnatq[i // 4][:, (i % 4) * L : (i % 4 + 1) * L] for i in range(MPC)]
            w12 = wpool.tile([128, 2 * H1 + 4 * H2], F16)
            nc.sync.dma_start(w12[:], d_w12[:])
            w1 = w12[:, 0 : 2 * H1]
            w2 = w12[:, 2 * H1 :]
            biasc = wpool.tile([128, 6], F32)
            nc.gpsimd.dma_start(biasc[:], d_bias[:])
            row = wpool.tile([1, 129], F16)
            nc.gpsimd.dma_start(row[:], d_row[:])

            ident = consts[:, C_IDENT : C_IDENT + 128]
            ones_col = consts[:, C_ONES : C_ONES + 1]

            # ---- XT = W_att.T-applied atoms: XT[d', a] -----------------
            ps_xt = pbig.tile([128, MPC * NPAD], F32, tag="big")
            nc.tensor.matmul(ps_xt[:], watt[:], atomT[:], start=True, stop=True)
            xt = work.tile([128, MPC * NPAD], F16)
            nc.vector.tensor_copy(xt[:], ps_xt[:])

            # ---- scores: S[a, l] per molecule, stacked 2/psum bank -----
            s_psums = []
            for s in range(NSTACK):
                ps_S = pbig.tile([128, L], F32, tag="big")
                s_psums.append(ps_S)
                for slot in range(2):
                    i = 2 * s + slot
                    nc.tensor.matmul(
                        ps_S[slot * NPAD : (slot + 1) * NPAD, :],
                        xt[:, i * NPAD : (i + 1) * NPAD],
                        protT[i],
                        start=True,
                        stop=True,
                    )

            # ---- S to SBUF; Wc; transposes; Wp -------------------------
            # wp col layout: 8*s + 2*j + slot  (molecule m = 2*s + slot)
            wc = work.tile([128, NSTACK], F32)
            wp = work.tile([128, 4 * MPC], F32)
            for s in range(NSTACK):
                s_sb = spool.tile([128, L], F16, tag="s_sb")
                nc.scalar.copy(s_sb[:], s_psums[s][:])
                nc.vector.reduce_max(wc[:, s : s + 1], s_psums[s][:], axis=AxX)
                ps_st = pq.tile([128, 4 * 128], F16, tag="q")
                for j in range(4):
                    nc.tensor.transpose(
                        ps_st[:, j * 128 : (j + 1) * 128],
                        s_sb[:, j * 128 : (j + 1) * 128],
                        ident,
                    )
                nc.vector.reduce_max(
                    wp[:, 8 * s : 8 * (s + 1)],
                    ps_st[:].rearrange("p (j g k) -> p j g k", j=4, k=NPAD),
                    axis=AxX,
                )
            wce = work.tile([128, NSTACK], F32)
            ew = work.tile([128, 4 * MPC], F16)
            for s in range(NSTACK):
                nc.scalar.activation(wce[:, s : s + 1], wc[:, s : s + 1], AF.Exp)
                nc.scalar.activation(
                    ew[:, 8 * s : 8 * (s + 1)], wp[:, 8 * s : 8 * (s + 1)], AF.Exp
                )

            # ---- denominators: Sc and t --------------------------------
            wcseg = work.tile([128, MPC], F16)
            for s in range(NSTACK):
                nc.vector.tensor_scalar_mul(
                    wcseg[:, 2 * s : 2 * s + 2],
                    in0=consts[:, C_IND + 2 * s : C_IND + 2 * s + 2],
                    scalar1=wce[:, s : s + 1],
                )
            ps_sc = ps.tile([1, MPC], F32, tag="sp")
            nc.tensor.matmul(ps_sc[:], ones_col, wcseg[:], start=True, stop=True)

            tpart = work.tile([128, MPC], F16)
            with nc.allow_low_precision(reason="sum of 4 fp16 values, 5e-4 rel"):
                nc.vector.reduce_sum(
                    tpart[:].rearrange("p (s sl) -> p s sl", sl=2),
                    ew[:].rearrange("p (s j sl) -> p s sl j", j=4, sl=2),
                    axis=AxX,
                )
            ps_t = ps.tile([1, MPC], F32, tag="sp")
            nc.tensor.matmul(ps_t[:], ones_col, tpart[:], start=True, stop=True)

            sct = work.tile([1, 2 * MPC], F16)
            nc.vector.tensor_copy(sct[:, :MPC], ps_sc[:])
            nc.vector.tensor_copy(sct[:, MPC:], ps_t[:])
            ps_bc = ps.tile([128, 2 * MPC], F32, tag="sp")
            nc.tensor.matmul(ps_bc[:], row[:, :128], sct[:], start=True, stop=True)
            inv = work.tile([128, 2 * MPC], F32)
            nc.vector.reciprocal(inv[:], ps_bc[:])

            # ---- pools -------------------------------------------------
            ps_ap = ps.tile([128, MPC], F32, tag="sp")
            for s in range(NSTACK):
                nc.tensor.matmul(
                    ps_ap[:, 2 * s : 2 * s + 2],
                    atomN[:, s, :],
                    wcseg[:, 2 * s : 2 * s + 2],
                    start=True,
                    stop=True,
                )
            # row-form pools packed 4 per PE column-group: molecule g*4+sl
            # accumulates in row 32*sl of psum tile prow[g].
            prows = []
            for g in range(2):
                ps_pr = pq.tile([128, 128], F32, tag="q")
                prows.append(ps_pr)
                for j in range(4):
                    for sl in range(4):
                        m = 4 * g + sl
                        ewc = 8 * (m // 2) + 2 * j + (m % 2)
                        nc.tensor.matmul(
                            ps_pr[32 * sl : 32 * sl + 1, :],
                            ew[:, ewc : ewc + 1],
                            pnat[m][:, j * 128 : (j + 1) * 128],
                            start=(j == 0),
                            stop=(j == 3),
                            tile_position=(0, 32 * sl),
                        )
            ps_ppT = []
            for g in range(2):
                pr_sb = work.tile([128, 128], F16, tag=f"prsb{g}")
                nc.scalar.copy(pr_sb[:], prows[g][:])
                ps_pt = pq.tile([128, 128], F16, tag="q")
                nc.tensor.transpose(ps_pt[:], pr_sb[:], ident)
                ps_ppT.append(ps_pt)

            htop = work.tile([128, MPC], F16)
            nc.vector.tensor_mul(htop[:], ps_ap[:], inv[:, :MPC])
            hbot = work.tile([128, MPC], F16)
            for g in range(2):
                nc.vector.tensor_mul(
                    hbot[:, 4 * g : 4 * g + 4],
                    ps_ppT[g][:].rearrange("p (a b) -> p b a", b=32)[:, 0, :],
                    inv[:, MPC + 4 * g : MPC + 4 * g + 4],
                )

            # ---- MLP ---------------------------------------------------
            h1 = work.tile([128, 4 * MPC], F16)
            for mc in range(4):
                ps_h1 = ps.tile([128, MPC], F32, tag="sp")
                nc.tensor.matmul(
                    ps_h1[:],
                    w1[:, mc * 128 : (mc + 1) * 128],
                    htop[:],
                    start=True,
                    stop=False,
                )
                nc.tensor.matmul(
                    ps_h1[:],
                    w1[:, H1 + mc * 128 : H1 + (mc + 1) * 128],
                    hbot[:],
                    start=False,
                    stop=True,
                )
                nc.scalar.activation(
                    h1[:, mc * MPC : (mc + 1) * MPC],
                    ps_h1[:],
                    AF.Relu,
                    bias=biasc[:, mc : mc + 1],
                )
            h2 = work.tile([128, 2 * MPC], F16)
            for mc2 in range(2):
                ps_h2 = ps.tile([128, MPC], F32, tag="sp")
                for kc in range(4):
                    nc.tensor.matmul(
                        ps_h2[:],
                        w2[:, kc * H2 + mc2 * 128 : kc * H2 + (mc2 + 1) * 128],
                        h1[:, kc * MPC : (kc + 1) * MPC],
                        start=(kc == 0),
                        stop=(kc == 3),
                    )
                nc.scalar.activation(
                    h2[:, mc2 * MPC : (mc2 + 1) * MPC],
                    ps_h2[:],
                    AF.Relu,
                    bias=biasc[:, 4 + mc2 : 4 + mc2 + 1],
                )
            ps_o = ps.tile([MPC, 1], F32, tag="sp")
            nc.tensor.matmul(
                ps_o[:], h2[:, :MPC], consts[:, C_WO : C_WO + 1], start=True, stop=False
            )
            nc.tensor.matmul(
                ps_o[:],
                h2[:, MPC : 2 * MPC],
                consts[:, C_WO + 1 : C_WO + 2],
                start=False,
                stop=False,
            )
            nc.tensor.matmul(
                ps_o[:], row[:, :MPC], row[:, 128:129], start=False, stop=True
            )
            y_sb = work.tile([MPC, 1], F32)
            nc.vector.tensor_copy(y_sb[:], ps_o[:])
            nc.sync.dma_start(d_y[:], y_sb[:])

    nc.compile()
    return nc


def _prep_inputs(atom_embed, protSeq_embed, atom_splits, W_att, W1, b1, W2, b2, Wo, bo):
    f16 = np.float16
    atom = np.asarray(atom_embed, dtype=np.float32)
    prot = np.asarray(protSeq_embed, dtype=np.float32)
    splits = np.asarray(atom_splits).astype(np.int64).ravel()
    order = np.argsort(splits, kind="stable")
    counts = np.bincount(splits, minlength=B)
    assert counts.max() <= NPAD, f"molecule with {counts.max()} atoms > NPAD={NPAD}"
    assert counts.min() >= 1, "empty molecule (reference produces NaN there)"
    offs = np.concatenate([[0], np.cumsum(counts)])

    atomP = np.empty((B, NPAD, D), np.float32)
    ind = np.zeros((B, NPAD), np.float32)
    for b in range(B):
        idx = order[offs[b] : offs[b + 1]]
        n = len(idx)
        atomP[b, :n] = atom[idx]
        atomP[b, n:] = atom[idx[0]]  # replicate a real atom: maxes stay exact
        ind[b, :n] = 1.0

    w_att = np.asarray(W_att, np.float32).astype(f16)  # [128, 128]
    w1h = (
        np.asarray(W1, np.float32)
        .reshape(2, 128, H1).transpose(1, 0, 2).reshape(128, 2 * H1).astype(f16)
    )
    w2h = (
        np.asarray(W2, np.float32)
        .reshape(4, 128, H2).transpose(1, 0, 2).reshape(128, 4 * H2).astype(f16)
    )
    b1c = np.asarray(b1, np.float32).reshape(4, 128).T
    b2c = np.asarray(b2, np.float32).reshape(2, 128).T
    biasc = np.zeros((128, 6), np.float32)
    biasc[:, 0:4] = b1c
    biasc[:, 4:6] = b2c
    woc = np.asarray(Wo, np.float32).reshape(2, 128).T.astype(f16)
    w12h = None  # built below
    row = np.zeros((1, 129), f16)
    row[0, :128] = 1.0
    row[0, 128] = np.asarray(bo, np.float32).ravel()[0]
    w12h = np.ascontiguousarray(np.concatenate([w1h, w2h], axis=1))

    in_maps = []
    for c in range(NCORES):
        sl = slice(c * MPC, (c + 1) * MPC)
        protT_c = np.ascontiguousarray(
            prot[sl].transpose(0, 2, 1).astype(f16)
        )  # [MPC, 128, L]
        pnat_c = np.ascontiguousarray(
            prot[sl].reshape(MPC, 4, 128, D).transpose(0, 2, 1, 3)
            .reshape(MPC, 128, L).astype(f16)
        )
        atomT_c = np.ascontiguousarray(atomP[sl].reshape(MPC * NPAD, D).T.astype(f16))
        atomN_c = np.ascontiguousarray(
            atomP[sl].reshape(NSTACK, 128, D).transpose(1, 0, 2)
            .reshape(128, NSTACK * D).astype(f16)
        )
        ind_c = np.zeros((128, MPC), f16)
        for m in range(MPC):
            s, slot = divmod(m, 2)
            ind_c[slot * NPAD : (slot + 1) * NPAD, m] = ind[c * MPC + m]
        consts = np.zeros((128, C_W), f16)
        consts[:, C_IDENT : C_IDENT + 128] = np.eye(128, dtype=f16)
        consts[:, C_IND : C_IND + MPC] = ind_c
        consts[:, C_ONES] = 1.0
        consts[:, C_WO : C_WO + 2] = woc
        im = {
            "atomw": np.ascontiguousarray(np.concatenate([atomT_c, w_att], axis=1)),
            "cons2": np.ascontiguousarray(np.concatenate([atomN_c, consts], axis=1)),
            "w12": w12h,
            "biasc": biasc,
            "row": row,
        }
        for q in range(4):
            im[f"protp{q}"] = np.ascontiguousarray(
                protT_c[2 * q : 2 * q + 2].transpose(1, 0, 2).reshape(128, 2 * L)
            )
        for q in range(2):
            im[f"pnatq{q}"] = np.ascontiguousarray(
                pnat_c[4 * q : 4 * q + 4].transpose(1, 0, 2).reshape(128, 4 * L)
            )
        in_maps.append(im)
    return in_maps


def kernel(atom_embed, protSeq_embed, atom_splits, W_att, W1, b1, W2, b2, Wo, bo,
           _trace=False):
    if "nc" not in _PROGRAM_CACHE:
        _PROGRAM_CACHE["nc"] = _build_program()
    nc = _PROGRAM_CACHE["nc"]
    in_maps = _prep_inputs(
        atom_embed, protSeq_embed, atom_splits, W_att, W1, b1, W2, b2, Wo, bo
    )
    res = run_bass_kernel_spmd(
        nc, in_maps, core_ids=list(range(NCORES)), trace=_trace
    )
    _PROGRAM_CACHE["last_result"] = res
    out = np.concatenate([res.results[c]["y"] for c in range(NCORES)], axis=0)
    return out.astype(np.float32)


# revision 19
# speedup vs baseline: 2.6887x; 1.0243x over previous
"""Trainium2 Bass kernel for nn_BiInteraction (segment softmax bi-interaction).

Strategy (data-parallel over molecules, 8 NeuronCores):
  - Each core owns 8 molecules (its contiguous slice of the batch) and gets
    its slice of protSeq_embed in two layouts (host-transposed protT for the
    score matmuls; natural-layout chunks for the attention pool), its atoms
    padded to 64 slots per molecule (pads are REPLICAS of a real atom, which
    keeps every max reduction exact without masks), an indicator matrix for
    segment sums, and the replicated MLP weights.
  - All matmul operands are fp16 (PSUM accumulation stays fp32): single-pass
    PE matmuls (fp32 runs LOW_HIGH double-pass at 4x the cost) and half the
    HBM traffic. End-to-end error vs the fp32 reference is ~1e-3.
  - Scores S[a, l] = (atom @ W_att) . prot[seg(a), l] are computed
    block-diagonally: one matmul per molecule (k = d = 128 contraction),
    two molecules stacked per PSUM bank.
  - Segment softmax over atoms:   Wc = exp(max_l S); Sc = 1^T (ind * Wc).
  - Residue softmax over protein: Wp = max_a S via PE transpose + grouped
    reduce; ew = exp(Wp); t = sum_l ew via ones-matmul.
  - Pools via matmuls (atoms / residues on the contraction partitions),
    normalization via a k=1 broadcast matmul + reciprocal, then the 3-layer
    MLP entirely on-chip per core ([256]->512->256->1 for its 8 molecules).

All shapes are static and identical across cores (single SPMD program);
per-core differences (counts, indicators, padding) live in the DMA'd data.
"""

import numpy as np

import concourse.bacc as bacc
import concourse.bass as bass
import concourse.tile as tile
from concourse import mybir
from concourse.bass_utils import run_bass_kernel_spmd

F32 = mybir.dt.float32
F16 = mybir.dt.float16
AxX = mybir.AxisListType.X
AF = mybir.ActivationFunctionType

A, L, D, B = 2048, 512, 128, 64
H1, H2 = 512, 256
NCORES = 8
MPC = B // NCORES            # molecules per core = 8
NPAD = 64                    # padded atom slots per molecule
NSTACK = MPC * NPAD // 128   # stacks of 128 padded atoms per core = 4

# fp16 consts tensor column layout
C_IDENT = 0        # [0, 128)   identity
C_IND = 128        # [128, 136) indicator, col = molecule
C_ONES = 136       # [136, 137) ones column
C_WO = 137         # [137, 139) Wo chunks
C_W = 139

_PROGRAM_CACHE = {}


def _build_program():
    nc = bacc.Bacc("TRN2", target_bir_lowering=False, debug=False)

    # atomw = atomT | watt (needed first); cons2 = atomN | consts
    AW_W = MPC * NPAD + D
    d_atomw = nc.dram_tensor("atomw", [128, AW_W], F16, kind="ExternalInput")
    CN_W = NSTACK * D + C_W
    d_cons2 = nc.dram_tensor("cons2", [128, CN_W], F16, kind="ExternalInput")
    d_protp = [
        nc.dram_tensor(f"protp{q}", [128, 2 * L], F16, kind="ExternalInput")
        for q in range(4)
    ]
    d_pnatq = [
        nc.dram_tensor(f"pnatq{q}", [128, 4 * L], F16, kind="ExternalInput")
        for q in range(2)
    ]
    d_w12 = nc.dram_tensor("w12", [128, 2 * H1 + 4 * H2], F16, kind="ExternalInput")
    d_bias = nc.dram_tensor("biasc", [128, 6], F32, kind="ExternalInput")
    d_row = nc.dram_tensor("row", [1, 129], F16, kind="ExternalInput")
    d_y = nc.dram_tensor("y", [MPC, 1], F32, kind="ExternalOutput")

    with tile.TileContext(nc) as tc:
        with (
            tc.tile_pool(name="weights", bufs=1) as wpool,
            tc.tile_pool(name="work", bufs=1) as work,
            tc.tile_pool(name="spool", bufs=4) as spool,
            tc.tile_pool(name="psum_big", bufs=3, space=bass.MemorySpace.PSUM) as pbig,
            tc.tile_pool(name="psum_q", bufs=3, space=bass.MemorySpace.PSUM) as pq,
            tc.tile_pool(name="psum_s", bufs=2, space=bass.MemorySpace.PSUM) as ps,
        ):
            # ---- loads: 3-queue issue, earliest-needed first -----------
            atomw = wpool.tile([128, AW_W], F16)
            nc.sync.dma_start(atomw[:], d_atomw[:])
            atomT = atomw[:, 0 : MPC * NPAD]
            watt = atomw[:, MPC * NPAD :]
            protp = []
            for q in range(4):
                pt = wpool.tile([128, 2 * L], F16, tag=f"protp{q}")
                protp.append(pt)
            nc.scalar.dma_start(protp[0][:], d_protp[0][:])
            nc.gpsimd.dma_start(protp[1][:], d_protp[1][:])
            nc.sync.dma_start(protp[2][:], d_protp[2][:])
            nc.scalar.dma_start(protp[3][:], d_protp[3][:])
            protT = [protp[i // 2][:, (i % 2) * L : (i % 2 + 1) * L] for i in range(MPC)]
            cons2 = wpool.tile([128, CN_W], F16)
            nc.gpsimd.dma_start(cons2[:], d_cons2[:])
            atomN = cons2[:, 0 : NSTACK * D].rearrange("p (s d) -> p s d", s=NSTACK)
            consts = cons2[:, NSTACK * D :]
            pnatq = []
            for q in range(2):
                pn = wpool.tile([128, 4 * L], F16, tag=f"pnatq{q}")
                eng = nc.sync if q % 2 == 0 else nc.scalar
                eng.dma_start(pn[:], d_pnatq[q][:])
                pnatq.append(pn)
            pnat = [pnatq[i // 4][:, (i % 4) * L : (i % 4 + 1) * L] for i in range(MPC)]
            w12 = wpool.tile([128, 2 * H1 + 4 * H2], F16)
            nc.gpsimd.dma_start(w12[:], d_w12[:])
            w1 = w12[:, 0 : 2 * H1]
            w2 = w12[:, 2 * H1 :]
            biasc = wpool.tile([128, 6], F32)
            nc.gpsimd.dma_start(biasc[:], d_bias[:])
            row = wpool.tile([1, 129], F16)
            nc.gpsimd.dma_start(row[:], d_row[:])

            ident = consts[:, C_IDENT : C_IDENT + 128]
            ones_col = consts[:, C_ONES : C_ONES + 1]

            # ---- XT = W_att.T-applied atoms: XT[d', a] -----------------
            ps_xt = pbig.tile([128, MPC * NPAD], F32, tag="big")
            nc.tensor.matmul(ps_xt[:], watt[:], atomT[:], start=True, stop=True)
            xt = work.tile([128, MPC * NPAD], F16)
            nc.vector.tensor_copy(xt[:], ps_xt[:])

            # ---- scores: S[a, l] per molecule, stacked 2/psum bank -----
            s_psums = []
            for s in range(NSTACK):
                ps_S = pbig.tile([128, L], F32, tag="big")
                s_psums.append(ps_S)
                for slot in range(2):
                    i = 2 * s + slot
                    nc.tensor.matmul(
                        ps_S[slot * NPAD : (slot + 1) * NPAD, :],
                        xt[:, i * NPAD : (i + 1) * NPAD],
                        protT[i],
                        start=True,
                        stop=True,
                    )

            # ---- S to SBUF; Wc; transposes; Wp -------------------------
            # wp col layout: 8*s + 2*j + slot  (molecule m = 2*s + slot)
            wc = work.tile([128, NSTACK], F32)
            wp = work.tile([128, 4 * MPC], F32)
            for s in range(NSTACK):
                s_sb = spool.tile([128, L], F16, tag="s_sb")
                nc.scalar.copy(s_sb[:], s_psums[s][:])
                nc.vector.reduce_max(wc[:, s : s + 1], s_psums[s][:], axis=AxX)
                ps_st = pq.tile([128, 4 * 128], F16, tag="q")
                for j in range(4):
                    nc.tensor.transpose(
                        ps_st[:, j * 128 : (j + 1) * 128],
                        s_sb[:, j * 128 : (j + 1) * 128],
                        ident,
                    )
                nc.vector.reduce_max(
                    wp[:, 8 * s : 8 * (s + 1)],
                    ps_st[:].rearrange("p (j g k) -> p j g k", j=4, k=NPAD),
                    axis=AxX,
                )
            wce = work.tile([128, NSTACK], F32)
            ew = work.tile([128, 4 * MPC], F16)
            for s in range(NSTACK):
                nc.scalar.activation(wce[:, s : s + 1], wc[:, s : s + 1], AF.Exp)
                nc.scalar.activation(
                    ew[:, 8 * s : 8 * (s + 1)], wp[:, 8 * s : 8 * (s + 1)], AF.Exp
                )

            # ---- denominators: Sc and t --------------------------------
            wcseg = work.tile([128, MPC], F16)
            for s in range(NSTACK):
                nc.vector.tensor_scalar_mul(
                    wcseg[:, 2 * s : 2 * s + 2],
                    in0=consts[:, C_IND + 2 * s : C_IND + 2 * s + 2],
                    scalar1=wce[:, s : s + 1],
                )
            ps_sc = ps.tile([1, MPC], F32, tag="sp")
            nc.tensor.matmul(ps_sc[:], ones_col, wcseg[:], start=True, stop=True)

            tpart = work.tile([128, MPC], F16)
            with nc.allow_low_precision(reason="sum of 4 fp16 values, 5e-4 rel"):
                nc.vector.reduce_sum(
                    tpart[:].rearrange("p (s sl) -> p s sl", sl=2),
                    ew[:].rearrange("p (s j sl) -> p s sl j", j=4, sl=2),
                    axis=AxX,
                )
            ps_t = ps.tile([1, MPC], F32, tag="sp")
            nc.tensor.matmul(ps_t[:], ones_col, tpart[:], start=True, stop=True)

            sct = work.tile([1, 2 * MPC], F16)
            nc.vector.tensor_copy(sct[:, :MPC], ps_sc[:])
            nc.vector.tensor_copy(sct[:, MPC:], ps_t[:])
            ps_bc = ps.tile([128, 2 * MPC], F32, tag="sp")
            nc.tensor.matmul(ps_bc[:], row[:, :128], sct[:], start=True, stop=True)
            inv = work.tile([128, 2 * MPC], F32)
            nc.vector.reciprocal(inv[:], ps_bc[:])

            # ---- pools -------------------------------------------------
            ps_ap = ps.tile([128, MPC], F32, tag="sp")
            for s in range(NSTACK):
                nc.tensor.matmul(
                    ps_ap[:, 2 * s : 2 * s + 2],
                    atomN[:, s, :],
                    wcseg[:, 2 * s : 2 * s + 2],
                    start=True,
                    stop=True,
                )
            # row-form pools packed 4 per PE column-group: molecule g*4+sl
            # accumulates in row 32*sl of psum tile prow[g].
            prows = []
            for g in range(2):
                ps_pr = pq.tile([128, 128], F32, tag="q")
                prows.append(ps_pr)
                for j in range(4):
                    for sl in range(4):
                        m = 4 * g + sl
                        ewc = 8 * (m // 2) + 2 * j + (m % 2)
                        nc.tensor.matmul(
                            ps_pr[32 * sl : 32 * sl + 1, :],
                            ew[:, ewc : ewc + 1],
                            pnat[m][:, j * 128 : (j + 1) * 128],
                            start=(j == 0),
                            stop=(j == 3),
                            tile_position=(0, 32 * sl),
                        )
            ps_ppT = []
            for g in range(2):
                pr_sb = work.tile([128, 128], F16, tag=f"prsb{g}")
                nc.scalar.copy(pr_sb[:], prows[g][:])
                ps_pt = pq.tile([128, 128], F16, tag="q")
                nc.tensor.transpose(ps_pt[:], pr_sb[:], ident)
                ps_ppT.append(ps_pt)

            htop = work.tile([128, MPC], F16)
            nc.vector.tensor_mul(htop[:], ps_ap[:], inv[:, :MPC])
            hbot = work.tile([128, MPC], F16)
            for g in range(2):
                nc.vector.tensor_mul(
                    hbot[:, 4 * g : 4 * g + 4],
                    ps_ppT[g][:].rearrange("p (a b) -> p b a", b=32)[:, 0, :],
                    inv[:, MPC + 4 * g : MPC + 4 * g + 4],
                )

            # ---- MLP ---------------------------------------------------
            h1 = work.tile([128, 4 * MPC], F16)
            for mc in range(4):
                ps_h1 = ps.tile([128, MPC], F32, tag="sp")
                nc.tensor.matmul(
                    ps_h1[:],
                    w1[:, mc * 128 : (mc + 1) * 128],
                    htop[:],
                    start=True,
                    stop=False,
                )
                nc.tensor.matmul(
                    ps_h1[:],
                    w1[:, H1 + mc * 128 : H1 + (mc + 1) * 128],
                    hbot[:],
                    start=False,
                    stop=True,
                )
                nc.scalar.activation(
                    h1[:, mc * MPC : (mc + 1) * MPC],
                    ps_h1[:],
                    AF.Relu,
                    bias=biasc[:, mc : mc + 1],
                )
            h2 = work.tile([128, 2 * MPC], F16)
            for mc2 in range(2):
                ps_h2 = ps.tile([128, MPC], F32, tag="sp")
                for kc in range(4):
                    nc.tensor.matmul(
                        ps_h2[:],
                        w2[:, kc * H2 + mc2 * 128 : kc * H2 + (mc2 + 1) * 128],
                        h1[:, kc * MPC : (kc + 1) * MPC],
                        start=(kc == 0),
                        stop=(kc == 3),
                    )
                nc.scalar.activation(
                    h2[:, mc2 * MPC : (mc2 + 1) * MPC],
                    ps_h2[:],
                    AF.Relu,
                    bias=biasc[:, 4 + mc2 : 4 + mc2 + 1],
                )
            ps_o = ps.tile([MPC, 1], F32, tag="sp")
            nc.tensor.matmul(
                ps_o[:], h2[:, :MPC], consts[:, C_WO : C_WO + 1], start=True, stop=False
            )
            nc.tensor.matmul(
                ps_o[:],
                h2[:, MPC : 2 * MPC],
                consts[:, C_WO + 1 : C_WO + 2],
                start=False,
                stop=False,
            )
            nc.tensor.matmul(
                ps_o[:], row[:, :MPC], row[:, 128:129], start=False, stop=True
            )
            y_sb = work.tile([MPC, 1], F32)
            nc.vector.tensor_copy(y_sb[:], ps_o[:])
            nc.sync.dma_start(d_y[:], y_sb[:])

    nc.compile()
    return nc


def _prep_inputs(atom_embed, protSeq_embed, atom_splits, W_att, W1, b1, W2, b2, Wo, bo):
    f16 = np.float16
    atom = np.asarray(atom_embed, dtype=np.float32)
    prot = np.asarray(protSeq_embed, dtype=np.float32)
    splits = np.asarray(atom_splits).astype(np.int64).ravel()
    order = np.argsort(splits, kind="stable")
    counts = np.bincount(splits, minlength=B)
    assert counts.max() <= NPAD, f"molecule with {counts.max()} atoms > NPAD={NPAD}"
    assert counts.min() >= 1, "empty molecule (reference produces NaN there)"
    offs = np.concatenate([[0], np.cumsum(counts)])

    atomP = np.empty((B, NPAD, D), np.float32)
    ind = np.zeros((B, NPAD), np.float32)
    for b in range(B):
        idx = order[offs[b] : offs[b + 1]]
        n = len(idx)
        atomP[b, :n] = atom[idx]
        atomP[b, n:] = atom[idx[0]]  # replicate a real atom: maxes stay exact
        ind[b, :n] = 1.0

    w_att = np.asarray(W_att, np.float32).astype(f16)  # [128, 128]
    w1h = (
        np.asarray(W1, np.float32)
        .reshape(2, 128, H1).transpose(1, 0, 2).reshape(128, 2 * H1).astype(f16)
    )
    w2h = (
        np.asarray(W2, np.float32)
        .reshape(4, 128, H2).transpose(1, 0, 2).reshape(128, 4 * H2).astype(f16)
    )
    b1c = np.asarray(b1, np.float32).reshape(4, 128).T
    b2c = np.asarray(b2, np.float32).reshape(2, 128).T
    biasc = np.zeros((128, 6), np.float32)
    biasc[:, 0:4] = b1c
    biasc[:, 4:6] = b2c
    woc = np.asarray(Wo, np.float32).reshape(2, 128).T.astype(f16)
    w12h = None  # built below
    row = np.zeros((1, 129), f16)
    row[0, :128] = 1.0
    row[0, 128] = np.asarray(bo, np.float32).ravel()[0]
    w12h = np.ascontiguousarray(np.concatenate([w1h, w2h], axis=1))

    in_maps = []
    for c in range(NCORES):
        sl = slice(c * MPC, (c + 1) * MPC)
        protT_c = np.ascontiguousarray(
            prot[sl].transpose(0, 2, 1).astype(f16)
        )  # [MPC, 128, L]
        pnat_c = np.ascontiguousarray(
            prot[sl].reshape(MPC, 4, 128, D).transpose(0, 2, 1, 3)
            .reshape(MPC, 128, L).astype(f16)
        )
        atomT_c = np.ascontiguousarray(atomP[sl].reshape(MPC * NPAD, D).T.astype(f16))
        atomN_c = np.ascontiguousarray(
            atomP[sl].reshape(NSTACK, 128, D).transpose(1, 0, 2)
            .reshape(128, NSTACK * D).astype(f16)
        )
        ind_c = np.zeros((128, MPC), f16)
        for m in range(MPC):
            s, slot = divmod(m, 2)
            ind_c[slot * NPAD : (slot + 1) * NPAD, m] = ind[c * MPC + m]
        consts = np.zeros((128, C_W), f16)
        consts[:, C_IDENT : C_IDENT + 128] = np.eye(128, dtype=f16)
        consts[:, C_IND : C_IND + MPC] = ind_c
        consts[:, C_ONES] = 1.0
        consts[:, C_WO : C_WO + 2] = woc
        im = {
            "atomw": np.ascontiguousarray(np.concatenate([atomT_c, w_att], axis=1)),
            "cons2": np.ascontiguousarray(np.concatenate([atomN_c, consts], axis=1)),
            "w12": w12h,
            "biasc": biasc,
            "row": row,
        }
        for q in range(4):
            im[f"protp{q}"] = np.ascontiguousarray(
                protT_c[2 * q : 2 * q + 2].transpose(1, 0, 2).reshape(128, 2 * L)
            )
        for q in range(2):
            im[f"pnatq{q}"] = np.ascontiguousarray(
                pnat_c[4 * q : 4 * q + 4].transpose(1, 0, 2).reshape(128, 4 * L)
            )
        in_maps.append(im)
    return in_maps


def kernel(atom_embed, protSeq_embed, atom_splits, W_att, W1, b1, W2, b2, Wo, bo,
           _trace=False):
    if "nc" not in _PROGRAM_CACHE:
        _PROGRAM_CACHE["nc"] = _build_program()
    nc = _PROGRAM_CACHE["nc"]
    in_maps = _prep_inputs(
        atom_embed, protSeq_embed, atom_splits, W_att, W1, b1, W2, b2, Wo, bo
    )
    res = run_bass_kernel_spmd(
        nc, in_maps, core_ids=list(range(NCORES)), trace=_trace
    )
    _PROGRAM_CACHE["last_result"] = res
    out = np.concatenate([res.results[c]["y"] for c in range(NCORES)], axis=0)
    return out.astype(np.float32)
